# revision 38
# baseline (speedup 1.0000x reference)
# Bidirectional Mamba block on 8 TRN2 NeuronCores — v7 (~684 us/core HW,
# vs ~702 us for v2; phase E runs with zero DVE bubbles).
#
# Sharding: core c = (b, dir, half): b = c // 4, dir = (c % 4) // 2,
# half = c % 2.  Each core runs one direction of one batch element for half
# (512) of the d_inner channels, computing the in_proj/conv/silu (u) for
# ALL 1024 channels so the x-projection needs no cross-core AllReduce.
# The only collective is the final 4-way ReduceScatter of the output
# projection partials.
#
# Structure (changes vs v2):
#  - head is one per-512-token-chunk pipeline: in_proj matmuls + causal
#    conv (3-col-overlap chunk tiles, no full-L xc staging) + x-projection
#    + dt=softplus (merged [128,1024] Exp/Ln) per chunk, so the proj
#    spills, B/C broadcasts and dt land long before phase E needs them;
#    the z projection (only needed by the pass-end gates) comes last.
#  - selective scan split into two L/2 chunks; the recurrent state crosses
#    the boundary via the scan's per-partition AP initial (per-state
#    scans, no Q-merge).  Phase F (out_proj) + fold eviction of the first
#    half run on PE/Act during the second half's scans.
#  - B/C broadcast quads [128, 4*1024] stream just-in-time two quads ahead
#    of the scan chain, B on the SP DMA queue / C on the Pool queue (a
#    single queue head-of-line-blocks).
#  - dA exp needs no memset/shift: chunk 0 uses initial=0.0 (dA[0] is
#    multiplied by 0), chunk 1 the carried state with dA = exp(A*dt).
#  - z gate kept resident in SBUF (no DRAM spill round-trip).
#  - LayerNorm via bn_stats/bn_aggr + fused scale-bias activation.
# Measured per-op costs and engine pitfalls (Pool/DVE SBUF port sharing,
# in-place TT pathology, STT stuck at 1x) are in the session notes.
import time
import numpy as np
from contextlib import ExitStack

import concourse.bass as bass
import concourse.mybir as mybir
import concourse.tile as tile
from concourse import bass_utils

F32 = mybir.dt.float32
BF16 = mybir.dt.bfloat16
I32 = mybir.dt.int32
AF = mybir.ActivationFunctionType
OP = mybir.AluOpType

B, L, D = 2, 2048, 512
DI, DS, DTR, DCONV = 1024, 16, 32, 4
NCORE = 8
DH = DI // 2            # d_inner channels per core (own half)
NGF = DI // 128         # 8 channel groups of 128 (full)
NG = DH // 128          # 4 own channel groups
NT = L // 128           # 16 token tiles
NC512 = L // 512        # 4 chunks of 512 along t

NH = 2                  # L-halves for the chunked scan
LH = L // NH            # 1024 tokens per half
QS = 4                  # states per quad tile
NQ = DS // QS           # 4 state quads
NPROJ = DTR + 2 * DS
# NOTE: offloading elementwise work to the Pool engine was measured to be
# a net loss — Pool TTs are ~7x slower than DVE *and* share SBUF ports
# with it, slowing concurrent DVE scans/TTs by 2-4x.


def _legalize_waits(nc, max_waits=1):
    """walrus's per-instruction sync-wait slots are limited (a Matmult with 2
    waits fails codegen).  Move excess waits onto a same-engine
    InstEventSemaphore inserted right before the instruction."""
    skip = ("InstEventSemaphore", "InstBassTrap",
            "InstTriggeredCopy", "InstNoOp",
            "InstDMAGatherAnt", "InstDMAScatterAddAnt", "InstTensorLoad",
            "InstTensorSave", "InstRegisterMove", "InstUnconditionalBranch")
    eng_map = {
        mybir.EngineType.DVE: nc.vector,
        mybir.EngineType.Activation: nc.scalar,
        mybir.EngineType.PE: nc.tensor,
        mybir.EngineType.Pool: nc.gpsimd,
        mybir.EngineType.SP: nc.sync,
    }
    n_split = 0
    for fn in nc.m.functions:
        for bb in fn.blocks:
            for target in list(bb.instructions):
                si = target.sync_info
                tname = type(target).__name__
                if (si is None or not si.on_wait
                        or len(si.on_wait) <= max_waits or tname in skip):
                    continue
                excess = list(si.on_wait[:-max_waits])
                keep = list(si.on_wait[-max_waits:])
                si.on_wait = keep
                # chain EventSemaphores, each carrying <= 2 waits
                for i0 in range(0, len(excess), 2):
                    ev = mybir.InstEventSemaphore(
                        name=nc.get_next_instruction_name(),
                        ins=[], outs=[],
                        sync_info=mybir.SyncInfo(
                            on_wait=excess[i0:i0 + 2], on_update=[]))
                    eng_map[target.engine].add_instruction(ev)
                    tail_bb = nc.m.functions[-1].blocks[-1]
                    evi = tail_bb.instructions[-1]
                    assert evi.name == ev.name
                    tail_insts = list(tail_bb.instructions)
                    tail_insts.pop()
                    tail_bb.instructions = tail_insts
                    insts = list(bb.instructions)
                    insts.insert(insts.index(target), evi)
                    bb.instructions = insts
                n_split += 1
    return n_split


def _build_nc(for_timeline=False):
    nc = bass.Bass("TRN2", target_bir_lowering=False, debug=False,
                   num_devices=NCORE)

    # ---------------- I/O declarations (per core) ----------------
    xpad_d = nc.dram_tensor("xpad", [D, DCONV - 1 + L], BF16,
                            kind="ExternalInput")
    winT_d = nc.dram_tensor("winT", [D, DI], BF16, kind="ExternalInput")
    wz_d = nc.dram_tensor("wz", [D, DH], BF16, kind="ExternalInput")
    wx_d = nc.dram_tensor("wx", [128, NGF * NPROJ], BF16,
                          kind="ExternalInput")
    wdt_d = nc.dram_tensor("wdt", [DTR, DH], BF16, kind="ExternalInput")
    wout_d = nc.dram_tensor("wout", [128, NG * D], BF16,
                            kind="ExternalInput")
    consts_d = nc.dram_tensor("consts", [128, 128], F32,
                              kind="ExternalInput")
    xres_d = nc.dram_tensor("xres", [L // 4, D], F32, kind="ExternalInput")
    ln_g_d = nc.dram_tensor("ln_g", [128, D], BF16, kind="ExternalInput")
    ln_b_d = nc.dram_tensor("ln_b", [128, D], BF16, kind="ExternalInput")
    sidx_d = nc.dram_tensor("sidx", [128, NT], I32, kind="ExternalInput")
    ident_d = nc.dram_tensor("ident", [128, 128], BF16,
                             kind="ExternalInput")
    out_d = nc.dram_tensor("out_shard", [L // 4, D], F32,
                           kind="ExternalOutput")

    quad_groups = [[0, 1, 2, 3], [4, 5, 6, 7]]
    # two-round pairwise ReduceScatter: round 1 reduces within each
    # direction pair (full f / full b per L-half), round 2 across
    # directions.  The 4-way "Mesh" RS is rank-asymmetric on HW (ranks
    # 0-1 take 52-73us, ranks 2-3 only 33-37us); two pairwise rounds
    # are symmetric.  Shard map: core with quad-rank r ends up with
    # logical quarter (0,2,1,3)[r] — mirrored on the host side.
    pair_groups_1 = [[0, 1], [2, 3], [4, 5], [6, 7]]
    pair_groups_2 = [[0, 2], [1, 3], [4, 6], [5, 7]]

    with tile.TileContext(nc) as tc:
        with ExitStack() as ctx:
            per = ctx.enter_context(tc.tile_pool(name="per", bufs=1))
            dram = ctx.enter_context(tc.tile_pool(name="dram", bufs=1,
                                                  space="DRAM"))

            out_bounce = dram.tile([L, D], BF16, tag="out_bounce",
                                   name="out_bounce")
            rs_out = dram.tile([L // 4, D], BF16, tag="rs_out",
                               name="rs_out")
            rs_mid = dram.tile([L // 2, D], BF16, tag="rs_mid",
                               name="rs_mid")
            bc_dram = dram.tile([2 * DS, L], BF16, tag="bc_dram",
                                name="bc_dram")

            # packed constants: [0:32 convw(8g x 4)][32:40 convb]
            # [40:44 b_dt][44:108 A][108:112 dskip][112:113 eps]
            cst = per.tile([128, 128], F32, tag="cst", name="cst")
            nc.sync.dma_start(cst[:], consts_d.ap())
            convw = cst[:, 0:32]
            convb = cst[:, 32:40]
            b_dt_sb = cst[:, 40:44]
            A_sb = cst[:, 44:108]
            dskip_sb = cst[:, 108:112]
            eps_sb = cst[:, 112:113]
            sidx_sb = per.tile([128, NT], I32, tag="sidx", name="sidx")
            nc.sync.dma_start(sidx_sb[:], sidx_d.ap())
            ident = per.tile([128, 128], BF16, tag="ident", name="ident")
            nc.sync.dma_start(ident[:], ident_d.ap())
            wout_sb = per.tile([128, NG * D], BF16, tag="wout",
                               name="wout")
            nc.sync.dma_start(wout_sb[:], wout_d.ap())

            # persistent activations
            dt = [per.tile([128, L], BF16, tag=f"dt{g}", name=f"dt{g}")
                  for g in range(NG)]
            du = [per.tile([128, L], BF16, tag=f"du{g}", name=f"du{g}")
                  for g in range(NG)]
            # sk holds the D_skip*u term (fold seed); the fold result is
            # evicted back over it per half
            sk = [per.tile([128, L], BF16, tag=f"sk{g}", name=f"sk{g}")
                  for g in range(NG)]
            z = [per.tile([128, L], BF16, tag=f"z{g}", name=f"z{g}")
                 for g in range(NG)]
            y_mm = [per.tile([128, L], BF16, tag=f"ymm{g}",
                             name=f"ymm{g}") for g in range(NG)]
            carry = [per.tile([128, DS], F32, tag=f"carry{g}",
                              name=f"carry{g}") for g in range(NG)]

            # B/C broadcast staging: [128, QS*LH] quad tiles, two rotating
            # slots each (per state-quad parity) so loads prefetch two
            # quads ahead of the scan chain.  Loaded per (half, pass).
            # Created before the scoped A/C/D pools (pool stack is LIFO).
            bcp = ctx.enter_context(tc.tile_pool(name="bcp", bufs=1))

            def load_bc_np(h, np_):
                """Returns (bq, cq) tiles for state-quad np_ of half h and
                emits their broadcast loads."""
                hsl = slice(h * LH, (h + 1) * LH)
                bqt = bcp.tile([128, QS * LH], BF16, tag=f"bq{np_ % 2}",
                               name=f"bq{h}_{np_}")
                cqt = bcp.tile([128, QS * LH], BF16, tag=f"cq{np_ % 2}",
                               name=f"cq{h}_{np_}")
                # B on the SP hardware-DGE queue, C on the Pool queue: two
                # queues halve the serial broadcast latency per quad
                for qi in range(QS):
                    n = np_ * QS + qi
                    nc.sync.dma_start(
                        bqt[:, qi * LH:(qi + 1) * LH],
                        bc_dram[n:n + 1, hsl].to_broadcast([128, LH]))
                    nc.gpsimd.dma_start(
                        cqt[:, qi * LH:(qi + 1) * LH],
                        bc_dram[DS + n:DS + n + 1,
                                hsl].to_broadcast([128, LH]))
                return bqt, cqt

            # ---------------- Phase A: in_proj + conv + silu ----------
            adctx = ExitStack()
            phu = adctx.enter_context(tc.tile_pool(name="phu", bufs=1))
            u = [phu.tile([128, L], BF16, tag=f"u{m}", name=f"u{m}")
                 for m in range(NGF)]
            with ExitStack() as actx:
                pha = actx.enter_context(tc.tile_pool(name="pha", bufs=1))
                scr = actx.enter_context(tc.tile_pool(name="scr", bufs=2))
                psA = actx.enter_context(
                    tc.tile_pool(name="psA", bufs=2, space="PSUM"))
                xpad_sb = [pha.tile([128, DCONV - 1 + L], BF16,
                                    tag=f"xp{k}", name=f"xp{k}")
                           for k in range(4)]
                winT_sb = [pha.tile([128, DI], BF16, tag=f"wi{k}",
                                    name=f"wi{k}") for k in range(4)]
                wz_sb = [pha.tile([128, DH], BF16, tag=f"wzk{k}",
                                  name=f"wzk{k}") for k in range(4)]
                for k in range(4):
                    sl = slice(k * 128, (k + 1) * 128)
                    nc.sync.dma_start(xpad_sb[k][:], xpad_d.ap()[sl, :])
                    nc.sync.dma_start(winT_sb[k][:], winT_d.ap()[sl, :])
                    nc.sync.dma_start(wz_sb[k][:], wz_d.ap()[sl, :])

                # Per-chunk pipeline (nn-outer): in_proj + conv for all 8
                # groups of a 512-token chunk, then that chunk's xproj and
                # (on odd chunks) the dt/du chain — so the proj spills, B/C
                # broadcasts and dt all land long before phase E needs them.
                psC = actx.enter_context(
                    tc.tile_pool(name="psC", bufs=2, space="PSUM"))
                psD = actx.enter_context(
                    tc.tile_pool(name="psD", bufs=2, space="PSUM"))
                phd = actx.enter_context(tc.tile_pool(name="phd", bufs=2))
                xccp = actx.enter_context(tc.tile_pool(name="xccp",
                                                       bufs=10))
                wx_sb = pha.tile([128, NGF * NPROJ], BF16, tag="wx",
                                 name="wx")
                nc.sync.dma_start(wx_sb[:], wx_d.ap())
                wdt_sb = pha.tile([DTR, DH], BF16, tag="wdt", name="wdt")
                nc.sync.dma_start(wdt_sb[:], wdt_d.ap())
                projT = pha.tile([NPROJ, L], BF16, tag="projT",
                                 name="projT")
                prev_xcc = {}
                for nn in range(NC512):
                    for m in range(NGF):
                        ps = psA.tile([128, 512], F32, tag="mmA", name="mmA")
                        for k in range(4):
                            nc.tensor.matmul(
                                out=ps[:],
                                lhsT=winT_sb[k][:, m * 128:(m + 1) * 128],
                                rhs=xpad_sb[k][:, DCONV - 1 + nn * 512:
                                               DCONV - 1 + (nn + 1) * 512],
                                start=(k == 0), stop=(k == 3))
                        # conv per 512-chunk with a 3-column overlap staged
                        # into each chunk tile (col k of xcc is
                        # xc[nn*512 + k - 3]); kills the 8 full-L xc tiles
                        xcc = xccp.tile([128, 515], BF16, tag="xcc",
                                        name="xcc")
                        nc.scalar.copy(xcc[:, 3:515], ps[:])
                        if nn == 0:
                            nc.vector.memset(xcc[:, 0:3], 0.0)
                        else:
                            nc.vector.tensor_copy(xcc[:, 0:3],
                                                  prev_xcc[m][:, 512:515])
                        prev_xcc[m] = xcc
                        # causal conv: u[t] = silu(b + sum_j w[3-j]*xc[t-j])
                        a_t = scr.tile([128, 512], BF16, tag="cacc",
                                       name="cacc")
                        nc.vector.tensor_scalar(
                            a_t[:], xcc[:, 3:515],
                            convw[:, m * DCONV + DCONV - 1:
                                  m * DCONV + DCONV],
                            convb[:, m:m + 1], OP.mult, OP.add)
                        for j in range(1, DCONV):
                            nc.vector.scalar_tensor_tensor(
                                a_t[:], xcc[:, 3 - j:515 - j],
                                convw[:, m * DCONV + DCONV - 1 - j:
                                      m * DCONV + DCONV - j],
                                a_t[:], OP.mult, OP.add)
                        nc.scalar.activation(
                            u[m][:, nn * 512:(nn + 1) * 512], a_t[:],
                            AF.Silu)
                    ps = psC.tile([NPROJ, 512], F32, tag="mmC", name="mmC")
                    for kg in range(NGF):
                        nc.tensor.matmul(
                            out=ps[:],
                            lhsT=wx_sb[:, kg * NPROJ:(kg + 1) * NPROJ],
                            rhs=u[kg][:, nn * 512:(nn + 1) * 512],
                            start=(kg == 0), stop=(kg == NGF - 1))
                    nc.scalar.copy(projT[:, nn * 512:(nn + 1) * 512], ps[:])
                    nc.sync.dma_start(
                        bc_dram[:, nn * 512:(nn + 1) * 512],
                        projT[DTR:NPROJ, nn * 512:(nn + 1) * 512])
                    if nn == 1:
                        # first half of bc_dram is complete: start the
                        # broadcast loads for the scan's first two state
                        # quads now so the DVE never waits at phase-E entry
                        bc_pre = {np_: load_bc_np(0, np_)
                                  for np_ in range(2)}
                    if nn % 2 == 1:
                        hh = nn // 2
                        for g in range(NG):
                            ps2 = psD.tile([128, 1024], F32, tag="mmD",
                                           name="mmD")
                            for ci in range(2):
                                c = nn - 1 + ci
                                nc.tensor.matmul(
                                    out=ps2[:, ci * 512:(ci + 1) * 512],
                                    lhsT=wdt_sb[:, g * 128:(g + 1) * 128],
                                    rhs=projT[0:DTR,
                                              c * 512:(c + 1) * 512],
                                    start=True, stop=True)
                            edt = phd.tile([128, 1024], F32, tag="edt",
                                           name="edt")
                            nc.scalar.activation(edt[:], ps2[:], AF.Exp,
                                                 bias=b_dt_sb[:, g:g + 1])
                            nc.scalar.activation(
                                dt[g][:, hh * LH:(hh + 1) * LH], edt[:],
                                AF.Ln, bias=1.0)
                        # du/sk per half right after its dt lands
                        for g in range(NG):
                            hs = slice(hh * LH, (hh + 1) * LH)
                            nc.vector.tensor_tensor(out=du[g][:, hs],
                                                    in0=dt[g][:, hs],
                                                    in1=u[g][:, hs],
                                                    op=OP.mult)
                            nc.vector.tensor_scalar(
                                sk[g][:, hs], u[g][:, hs],
                                dskip_sb[:, g:g + 1], None, OP.mult)
                # z (own half) -> silu -> z[g]; only needed by the
                # pass-end gates, so emitted after the dt/du chain
                for g in range(NG):
                    for nn in range(NC512):
                        ps = psA.tile([128, 512], F32, tag="mmA", name="mmA")
                        for k in range(4):
                            nc.tensor.matmul(
                                out=ps[:],
                                lhsT=wz_sb[k][:, g * 128:(g + 1) * 128],
                                rhs=xpad_sb[k][:, DCONV - 1 + nn * 512:
                                               DCONV - 1 + (nn + 1) * 512],
                                start=(k == 0), stop=(k == 3))
                        nc.scalar.activation(
                            z[g][:, nn * 512:(nn + 1) * 512], ps[:],
                            AF.Silu)
            adctx.close()  # frees the u tiles before phase E

            # ---------------- Phase E: selective scan ----------------
            with ExitStack() as ectx:
                trb = ectx.enter_context(tc.tile_pool(name="trb", bufs=1))
                # dA double-buffered so the Act exps prefetch a full
                # (g, state-quad) iteration ahead of the DVE scan chain
                dApool = ectx.enter_context(tc.tile_pool(name="dAp",
                                                         bufs=2))
                psY = ectx.enter_context(
                    tc.tile_pool(name="psY", bufs=1, space="PSUM"))
                psF = ectx.enter_context(
                    tc.tile_pool(name="psF", bufs=4, space="PSUM"))
                osb_pool = ectx.enter_context(tc.tile_pool(name="osb",
                                                           bufs=4))

                first_pass_pre = bc_pre
                for h in range(NH):
                    hsl = slice(h * LH, (h + 1) * LH)
                    for gp in range(NG // 2):
                        gs = [2 * gp, 2 * gp + 1]
                        # B/C quad tiles for this pass: the first two come
                        # either from the C/D-phase prefetch (very first
                        # pass) or are loaded now; later quads are loaded
                        # two ahead of the scan chain inside the np loop.
                        bct = first_pass_pre if first_pass_pre else {
                            np_: load_bc_np(h, np_) for np_ in range(2)}
                        first_pass_pre = None
                        ytiles = {g: [psY.tile([128, 512], F32,
                                               tag=f"psY{g % 2}_{q}",
                                               name=f"psY{g}_{h}_{q}")
                                      for q in range(2)] for g in gs}
                        # seed the fold chains with the D_skip*u term
                        for g in gs:
                            for q in range(2):
                                nc.tensor.matmul(
                                    out=ytiles[g][q][:],
                                    lhsT=ident[:],
                                    rhs=sk[g][:, h * LH + q * 512:
                                              h * LH + (q + 1) * 512],
                                    start=True, stop=False)

                        def w_mult(dst, g, bqt, eng):
                            eng.tensor_tensor(
                                out=dst[:].rearrange(
                                    "p (q l) -> p q l", q=QS),
                                in0=du[g][:, hsl].rearrange(
                                    "p (q l) -> p q l",
                                    q=1).to_broadcast([128, QS, LH]),
                                in1=bqt[:].rearrange(
                                    "p (q l) -> p q l", q=QS),
                                op=OP.mult)

                        for np_ in range(NQ):
                            if np_ + 2 < NQ:
                                bct[np_ + 2] = load_bc_np(h, np_ + 2)
                            for g in gs:
                                dA = dApool.tile([128, QS * LH], BF16,
                                                 tag="dAp", name="dA")
                                for qi in range(QS):
                                    n = np_ * QS + qi
                                    nidx = g * DS + n
                                    nc.scalar.activation(
                                        dA[:, qi * LH:(qi + 1) * LH],
                                        dt[g][:, hsl], AF.Exp,
                                        scale=A_sb[:, nidx:nidx + 1])
                                w_t = trb.tile([128, QS * LH], BF16,
                                               tag="w", name="w")
                                w_mult(w_t, g, bct[np_][0], nc.vector)
                                h_t = trb.tile([128, QS * LH], BF16,
                                               tag="h", name="h")
                                for qi in range(QS):
                                    n = np_ * QS + qi
                                    init = (0.0 if h == 0
                                            else carry[g][:, n:n + 1])
                                    nc.vector.tensor_tensor_scan(
                                        h_t[:, qi * LH:(qi + 1) * LH],
                                        dA[:, qi * LH:(qi + 1) * LH],
                                        w_t[:, qi * LH:(qi + 1) * LH],
                                        init, OP.mult, OP.add)
                                if h == 0:
                                    # save the carried state (last column
                                    # of each state's chunk)
                                    nc.scalar.copy(
                                        carry[g][:, np_ * QS:
                                                 (np_ + 1) * QS].rearrange(
                                            "p (q o) -> p q o", o=1),
                                        h_t[:].rearrange(
                                            "p (q l) -> p q l",
                                            q=QS)[:, :, LH - 1:LH])
                                p_t = trb.tile([128, QS * LH], BF16,
                                               tag="p", name="p")
                                nc.vector.tensor_tensor(
                                    out=p_t[:], in0=h_t[:],
                                    in1=bct[np_][1][:],
                                    op=OP.mult)
                                last = (np_ == NQ - 1)
                                for qi in range(QS):
                                    for q in range(2):
                                        nc.tensor.matmul(
                                            out=ytiles[g][q][:],
                                            lhsT=ident[:],
                                            rhs=p_t[:, qi * LH + q * 512:
                                                    qi * LH + (q + 1) * 512],
                                            start=False,
                                            stop=(last and qi == QS - 1))
                        # eviction + silu(z) gate for this pass's 2 groups
                        for g in gs:
                            for q in range(2):
                                nc.scalar.copy(
                                    sk[g][:, h * LH + q * 512:
                                          h * LH + (q + 1) * 512],
                                    ytiles[g][q][:])
                            nc.vector.tensor_tensor(
                                out=y_mm[g][:, hsl], in0=sk[g][:, hsl],
                                in1=z[g][:, hsl], op=OP.mult)

                    # ---- Phase F for this half: out_proj + scatter ----
                    for j in range(L // NH // 128):
                        tt = h * (LH // 128) + j
                        ps = psF.tile([128, D], F32, tag="mmF", name="mmF")
                        for g in range(NG):
                            nc.tensor.matmul(
                                out=ps[:],
                                lhsT=y_mm[g][:, tt * 128:(tt + 1) * 128],
                                rhs=wout_sb[:, g * D:(g + 1) * D],
                                start=(g == 0), stop=(g == NG - 1))
                        o_sb = osb_pool.tile([128, D], BF16, tag="osb",
                                             name="osb")
                        nc.scalar.copy(o_sb[:], ps[:])
                        nc.gpsimd.indirect_dma_start(
                            out=out_bounce.opt(),
                            out_offset=bass.IndirectOffsetOnAxis(
                                ap=sidx_sb[:, tt:tt + 1], axis=0),
                            in_=o_sb[:],
                            in_offset=None)

            # -------- Phase G: ReduceScatter + LN/SiLU/residual --------
            with ExitStack() as gctx:
                phg2 = gctx.enter_context(tc.tile_pool(name="phg2",
                                                       bufs=1))
                # prefetch the residual/LN params before the collective so
                # their DMAs overlap the ReduceScatter wait
                xres_sb = phg2.tile([128, 4 * D], F32, tag="xres",
                                    name="xres")
                for qq in range(4):
                    nc.sync.dma_start(xres_sb[:, qq * D:(qq + 1) * D],
                                      xres_d.ap()[qq * 128:(qq + 1) * 128,
                                                  :])
                lng_sb = phg2.tile([128, D], BF16, tag="lng", name="lng")
                nc.sync.dma_start(lng_sb[:], ln_g_d.ap())
                lnb_sb = phg2.tile([128, D], BF16, tag="lnb", name="lnb")
                nc.sync.dma_start(lnb_sb[:], ln_b_d.ap())
                if for_timeline:
                    nc.sync.dma_start(rs_out[:], out_bounce[0:L // 4, :])
                else:
                    nc.gpsimd.collective_compute(
                        "ReduceScatter", OP.add,
                        replica_groups=pair_groups_1,
                        ins=[out_bounce.opt()], outs=[rs_mid.opt()])
                    nc.gpsimd.collective_compute(
                        "ReduceScatter", OP.add,
                        replica_groups=pair_groups_2,
                        ins=[rs_mid.opt()], outs=[rs_out.opt()])
                rs_bf = phg2.tile([128, 4 * D], BF16, tag="rsb",
                                  name="rsb")
                for qq in range(4):
                    nc.sync.dma_start(rs_bf[:, qq * D:(qq + 1) * D],
                                      rs_out[qq * 128:(qq + 1) * 128, :])

                for qq in range(4):
                    rtile = rs_bf[:, qq * D:(qq + 1) * D]
                    st = phg2.tile([128, 8], F32, tag=f"st{qq}",
                                   name=f"st{qq}")
                    nc.vector.bn_stats(st[:, 0:6], rtile)
                    nc.vector.bn_aggr(st[:, 6:8], st[:, 0:6])
                    mu = st[:, 6:7]
                    var = st[:, 7:8]
                    std = phg2.tile([128, 1], F32, tag=f"std{qq}",
                                    name=f"std{qq}")
                    nc.scalar.activation(std[:], var[:], AF.Sqrt,
                                         bias=eps_sb[:, 0:1])
                    rstd = phg2.tile([128, 1], F32, tag=f"rstd{qq}",
                                     name=f"rstd{qq}")
                    nc.vector.reciprocal(rstd[:], std[:])
                    nmr = phg2.tile([128, 1], F32, tag=f"nmr{qq}",
                                    name=f"nmr{qq}")
                    nc.vector.tensor_scalar(nmr[:], mu[:], rstd[:, 0:1],
                                            -1.0, OP.mult, OP.mult)
                    s1 = phg2.tile([128, D], BF16, tag=f"s1{qq}",
                                   name=f"s1{qq}")
                    nc.scalar.activation(s1[:], rtile, AF.Identity,
                                         scale=rstd[:, 0:1],
                                         bias=nmr[:, 0:1])
                    s2 = phg2.tile([128, D], BF16, tag=f"s2{qq}",
                                   name=f"s2{qq}")
                    nc.vector.tensor_tensor(out=s2[:], in0=s1[:],
                                            in1=lng_sb[:], op=OP.mult)
                    s3 = phg2.tile([128, D], BF16, tag=f"s3{qq}",
                                   name=f"s3{qq}")
                    nc.vector.tensor_tensor(out=s3[:], in0=s2[:],
                                            in1=lnb_sb[:], op=OP.add)
                    sil = phg2.tile([128, D], F32, tag=f"sil{qq}",
                                    name=f"sil{qq}")
                    nc.scalar.activation(sil[:], s3[:], AF.Silu)
                    fin = phg2.tile([128, D], F32, tag=f"fin{qq}",
                                    name=f"fin{qq}")
                    nc.vector.tensor_tensor(
                        out=fin[:], in0=sil[:],
                        in1=xres_sb[:, qq * D:(qq + 1) * D], op=OP.add)
                    nc.sync.dma_start(
                        out_d.ap()[qq * 128:(qq + 1) * 128, :], fin[:])

    _legalize_waits(nc)
    return nc


_NC_CACHE = {}


def _get_nc():
    if "nc" not in _NC_CACHE:
        _NC_CACHE["nc"] = _build_nc()
    return _NC_CACHE["nc"]


def _pg(a, ngroups):
    """[ngroups*128, k] -> [128, ngroups*k] partition-major regroup."""
    k = a.shape[1]
    return np.ascontiguousarray(
        a.reshape(ngroups, 128, k).transpose(1, 0, 2)).reshape(
            128, ngroups * k)


def _prep_in_maps(inputs):
    import ml_dtypes
    BF = ml_dtypes.bfloat16
    x = np.asarray(inputs["x"], np.float32)
    ln_g = np.asarray(inputs["ln_g"], np.float32)
    ln_b = np.asarray(inputs["ln_b"], np.float32)
    ln_gb = np.broadcast_to(ln_g[None, :], (128, D)).copy()
    ln_bb = np.broadcast_to(ln_b[None, :], (128, D)).copy()

    in_maps = []
    for c in range(NCORE):
        b, dr, half = c // 4, (c % 4) // 2, c % 2
        pfx = "fw_" if dr == 0 else "bw_"
        W_in = np.asarray(inputs[pfx + "W_in"], np.float32)
        convw = np.asarray(inputs[pfx + "conv_w"], np.float32)
        convb = np.asarray(inputs[pfx + "conv_b"], np.float32)
        W_x = np.asarray(inputs[pfx + "W_xproj"], np.float32)
        W_dt = np.asarray(inputs[pfx + "W_dt"], np.float32)
        b_dt = np.asarray(inputs[pfx + "b_dt"], np.float32)
        A_log = np.asarray(inputs[pfx + "A_log"], np.float32)
        dskip = np.asarray(inputs[pfx + "D_skip"], np.float32)
        W_out = np.asarray(inputs[pfx + "W_out"], np.float32)

        h0 = half * DH
        oh0 = (1 - half) * DH
        perm = np.concatenate([np.arange(h0, h0 + DH),
                               np.arange(oh0, oh0 + DH)])
        own = perm[:DH]

        xb = x[b] if dr == 0 else x[b][::-1]
        xpad = np.zeros((D, DCONV - 1 + L), np.float32)
        xpad[:, DCONV - 1:] = xb.T

        q = (0, 2, 1, 3)[c % 4]
        t = np.arange(NT * 128).reshape(NT, 128)
        rows = t if dr == 0 else (L - 1) - t

        consts = np.zeros((128, 128), np.float32)
        consts[:, 0:32] = _pg(convw[perm], NGF)
        consts[:, 32:40] = _pg(convb[perm][:, None], NGF)
        consts[:, 40:44] = _pg(b_dt[own][:, None], NG)
        consts[:, 44:108] = _pg(-np.exp(A_log[own]), NG)
        consts[:, 108:112] = _pg(dskip[own][:, None], NG)
        consts[:, 112] = 1e-5

        wx = np.empty((128, NGF * NPROJ), np.float32)
        for kg in range(NGF):
            ch = perm[kg * 128:(kg + 1) * 128]
            wx[:, kg * NPROJ:(kg + 1) * NPROJ] = W_x[:, ch].T

        m = {
            "xpad": xpad.astype(BF),
            "winT": np.ascontiguousarray(
                W_in[:DI][perm].T).astype(BF),
            "wz": np.ascontiguousarray(
                W_in[DI + h0:DI + h0 + DH].T).astype(BF),
            "wx": wx.astype(BF),
            "wdt": np.ascontiguousarray(W_dt[own].T).astype(BF),
            "wout": _pg(0.5 * W_out[:, own].T, NG).astype(BF),
            "consts": consts,
            "xres": np.ascontiguousarray(
                x[b][q * (L // 4):(q + 1) * (L // 4)]),
            "ln_g": ln_gb.astype(BF),
            "ln_b": ln_bb.astype(BF),
            "sidx": np.ascontiguousarray(rows.T.astype(np.int32)),
            "ident": np.eye(128, dtype=np.float32).astype(BF),
        }
        in_maps.append(m)
    return in_maps


def _assemble(results):
    out = np.empty((B, L, D), np.float32)
    for b in range(B):
        for r in range(4):
            q = (0, 2, 1, 3)[r]
            out[b, q * (L // 4):(q + 1) * (L // 4)] = \
                results[4 * b + r]["out_shard"]
    return out


def _run(inputs, trace=False, **kw):
    nc = _get_nc()
    in_maps = _prep_in_maps(inputs)
    res = bass_utils.run_bass_kernel_spmd(
        nc, in_maps, core_ids=list(range(NCORE)), trace=trace, **kw)
    return _assemble(res.results), res


def _timed_run(inputs, iters=10):
    """Run once for outputs, then time repeated executions of the jitted
    sharded body (no donation; inputs resident on device)."""
    import jax
    import numpy as _np
    from jax.sharding import Mesh, PartitionSpec, NamedSharding
    from jax.experimental.shard_map import shard_map
    import concourse.bass2jax as bass2jax
    import concourse.mybir as _mybir

    nc = _get_nc()
    in_maps = _prep_in_maps(inputs)
    bass2jax.install_neuronx_cc_hook()

    partition_name = (nc.partition_id_tensor.name
                      if nc.partition_id_tensor else None)
    in_names, out_names, out_avals, zero_outs = [], [], [], []
    for alloc in nc.m.functions[0].allocations:
        if not isinstance(alloc, _mybir.MemoryLocationSet):
            continue
        name = alloc.memorylocations[0].name
        if alloc.kind == "ExternalInput":
            if name != partition_name:
                in_names.append(name)
        elif alloc.kind == "ExternalOutput":
            shape = tuple(alloc.tensor_shape)
            dtype = _mybir.dt.np(alloc.dtype)
            out_names.append(name)
            out_avals.append(jax.core.ShapedArray(shape, dtype))
            zero_outs.append(_np.zeros(shape, dtype))
    n_params = len(in_names)
    n_outs = len(out_avals)
    all_in_names = list(in_names) + list(out_names)
    if partition_name is not None:
        all_in_names.append(partition_name)

    def _body(*args):
        operands = list(args)
        if partition_name is not None:
            operands.append(bass2jax.partition_id_tensor())
        outs = bass2jax._bass_exec_p.bind(
            *operands,
            out_avals=tuple(out_avals),
            in_names=tuple(all_in_names),
            out_names=tuple(out_names),
            lowering_input_output_aliases=(),
            sim_require_finite=True,
            sim_require_nnan=True,
            nc=nc,
        )
        return tuple(outs)

    devices = jax.devices()[:NCORE]
    mesh = Mesh(_np.asarray(devices), ("core",))
    in_specs = (PartitionSpec("core"),) * (n_params + n_outs)
    out_specs = (PartitionSpec("core"),) * n_outs
    donate = tuple(range(n_params, n_params + n_outs))
    sharded = jax.jit(
        shard_map(_body, mesh=mesh, in_specs=in_specs, out_specs=out_specs,
                  check_rep=False),
        donate_argnums=donate, keep_unused=True)

    sh = NamedSharding(mesh, PartitionSpec("core"))
    concat_in = [
        jax.device_put(_np.concatenate(
            [_np.asarray(in_maps[c][nm]) for c in range(NCORE)], axis=0), sh)
        for nm in in_names
    ]
    def make_zeros():
        return [jax.device_put(
            _np.zeros((NCORE * z.shape[0], *z.shape[1:]), z.dtype), sh)
            for z in zero_outs]

    jax.block_until_ready(concat_in)
    z0 = make_zeros()
    jax.block_until_ready(z0)
    out_arrs = jax.block_until_ready(sharded(*concat_in, *z0))
    results = [
        {nm: _np.asarray(out_arrs[i]).reshape(NCORE, *out_avals[i].shape)[c]
         for i, nm in enumerate(out_names)}
        for c in range(NCORE)
    ]
    output = _assemble(results)

    zsets = [make_zeros() for _ in range(iters)]
    jax.block_until_ready(zsets)
    best = None
    for zi in zsets:
        t1 = time.perf_counter()
        jax.block_until_ready(sharded(*concat_in, *zi))
        dt_s = time.perf_counter() - t1
        best = dt_s if best is None else min(best, dt_s)
    return output, int(best * 1e9)


def kernel(**inputs):
    out, _ = _run(inputs)
    return out


# revision 39
# speedup vs baseline: 1.2305x; 1.2305x over previous
# Bidirectional Mamba block on 8 TRN2 NeuronCores — v7 (~684 us/core HW,
# vs ~702 us for v2; phase E runs with zero DVE bubbles).
#
# Sharding: core c = (b, dir, half): b = c // 4, dir = (c % 4) // 2,
# half = c % 2.  Each core runs one direction of one batch element for half
# (512) of the d_inner channels, computing the in_proj/conv/silu (u) for
# ALL 1024 channels so the x-projection needs no cross-core AllReduce.
# The only collective is the final 4-way ReduceScatter of the output
# projection partials.
#
# Structure (changes vs v2):
#  - head is one per-512-token-chunk pipeline: in_proj matmuls + causal
#    conv (3-col-overlap chunk tiles, no full-L xc staging) + x-projection
#    + dt=softplus (merged [128,1024] Exp/Ln) per chunk, so the proj
#    spills, B/C broadcasts and dt land long before phase E needs them;
#    the z projection (only needed by the pass-end gates) comes last.
#  - selective scan split into two L/2 chunks; the recurrent state crosses
#    the boundary via the scan's per-partition AP initial (per-state
#    scans, no Q-merge).  Phase F (out_proj) + fold eviction of the first
#    half run on PE/Act during the second half's scans.
#  - B/C broadcast quads [128, 4*1024] stream just-in-time two quads ahead
#    of the scan chain, B on the SP DMA queue / C on the Pool queue (a
#    single queue head-of-line-blocks).
#  - dA exp needs no memset/shift: chunk 0 uses initial=0.0 (dA[0] is
#    multiplied by 0), chunk 1 the carried state with dA = exp(A*dt).
#  - z gate kept resident in SBUF (no DRAM spill round-trip).
#  - LayerNorm via bn_stats/bn_aggr + fused scale-bias activation.
# Measured per-op costs and engine pitfalls (Pool/DVE SBUF port sharing,
# in-place TT pathology, STT stuck at 1x) are in the session notes.
import time
import numpy as np
from contextlib import ExitStack

import concourse.bass as bass
import concourse.mybir as mybir
import concourse.tile as tile
from concourse import bass_utils

F32 = mybir.dt.float32
BF16 = mybir.dt.bfloat16
I32 = mybir.dt.int32
AF = mybir.ActivationFunctionType
OP = mybir.AluOpType

B, L, D = 2, 2048, 512
DI, DS, DTR, DCONV = 1024, 16, 32, 4
NCORE = 8
DH = DI // 2            # d_inner channels per core (own half)
NGF = DI // 128         # 8 channel groups of 128 (full)
NG = DH // 128          # 4 own channel groups
NT = L // 128           # 16 token tiles
NC512 = L // 512        # 4 chunks of 512 along t

NH = 2                  # L-halves for the chunked scan
LH = L // NH            # 1024 tokens per half
QS = 4                  # states per quad tile
NQ = DS // QS           # 4 state quads
NPROJ = DTR + 2 * DS
# NOTE: offloading elementwise work to the Pool engine was measured to be
# a net loss — Pool TTs are ~7x slower than DVE *and* share SBUF ports
# with it, slowing concurrent DVE scans/TTs by 2-4x.


def _legalize_waits(nc, max_waits=1):
    """walrus's per-instruction sync-wait slots are limited (a Matmult with 2
    waits fails codegen).  Move excess waits onto a same-engine
    InstEventSemaphore inserted right before the instruction."""
    skip = ("InstEventSemaphore", "InstBassTrap",
            "InstTriggeredCopy", "InstNoOp",
            "InstDMAGatherAnt", "InstDMAScatterAddAnt", "InstTensorLoad",
            "InstTensorSave", "InstRegisterMove", "InstUnconditionalBranch")
    eng_map = {
        mybir.EngineType.DVE: nc.vector,
        mybir.EngineType.Activation: nc.scalar,
        mybir.EngineType.PE: nc.tensor,
        mybir.EngineType.Pool: nc.gpsimd,
        mybir.EngineType.SP: nc.sync,
    }
    n_split = 0
    for fn in nc.m.functions:
        for bb in fn.blocks:
            for target in list(bb.instructions):
                si = target.sync_info
                tname = type(target).__name__
                if (si is None or not si.on_wait
                        or len(si.on_wait) <= max_waits or tname in skip):
                    continue
                excess = list(si.on_wait[:-max_waits])
                keep = list(si.on_wait[-max_waits:])
                si.on_wait = keep
                # chain EventSemaphores, each carrying <= 2 waits
                for i0 in range(0, len(excess), 2):
                    ev = mybir.InstEventSemaphore(
                        name=nc.get_next_instruction_name(),
                        ins=[], outs=[],
                        sync_info=mybir.SyncInfo(
                            on_wait=excess[i0:i0 + 2], on_update=[]))
                    eng_map[target.engine].add_instruction(ev)
                    tail_bb = nc.m.functions[-1].blocks[-1]
                    evi = tail_bb.instructions[-1]
                    assert evi.name == ev.name
                    tail_insts = list(tail_bb.instructions)
                    tail_insts.pop()
                    tail_bb.instructions = tail_insts
                    insts = list(bb.instructions)
                    insts.insert(insts.index(target), evi)
                    bb.instructions = insts
                n_split += 1
    return n_split


def _build_nc(for_timeline=False):
    nc = bass.Bass("TRN2", target_bir_lowering=False, debug=False,
                   num_devices=NCORE)

    # ---------------- I/O declarations (per core) ----------------
    xpad_d = nc.dram_tensor("xpad", [D, DCONV - 1 + L], BF16,
                            kind="ExternalInput")
    winT_d = nc.dram_tensor("winT", [D, DI], BF16, kind="ExternalInput")
    wz_d = nc.dram_tensor("wz", [D, DH], BF16, kind="ExternalInput")
    wx_d = nc.dram_tensor("wx", [128, NGF * NPROJ], BF16,
                          kind="ExternalInput")
    wdt_d = nc.dram_tensor("wdt", [DTR, DH], BF16, kind="ExternalInput")
    wout_d = nc.dram_tensor("wout", [128, NG * D], BF16,
                            kind="ExternalInput")
    consts_d = nc.dram_tensor("consts", [128, 128], F32,
                              kind="ExternalInput")
    xres_d = nc.dram_tensor("xres", [L // 4, D], F32, kind="ExternalInput")
    ln_g_d = nc.dram_tensor("ln_g", [128, D], BF16, kind="ExternalInput")
    ln_b_d = nc.dram_tensor("ln_b", [128, D], BF16, kind="ExternalInput")
    sidx_d = nc.dram_tensor("sidx", [128, NT], I32, kind="ExternalInput")
    ident_d = nc.dram_tensor("ident", [128, 128], BF16,
                             kind="ExternalInput")
    out_d = nc.dram_tensor("out_shard", [L // 4, D], F32,
                           kind="ExternalOutput")

    # NOTE: the 4-way "Mesh" RS is rank-asymmetric on HW (ranks 0-1 take
    # 52-73us, ranks 2-3 only 33-37us), but splitting it into two
    # pairwise ReduceScatter rounds measured ~150us SLOWER overall —
    # per-collective fixed overhead dominates.  Keep the single 4-way op.
    quad_groups = [[0, 1, 2, 3], [4, 5, 6, 7]]

    with tile.TileContext(nc) as tc:
        with ExitStack() as ctx:
            per = ctx.enter_context(tc.tile_pool(name="per", bufs=1))
            dram = ctx.enter_context(tc.tile_pool(name="dram", bufs=1,
                                                  space="DRAM"))

            out_bounce = dram.tile([L, D], BF16, tag="out_bounce",
                                   name="out_bounce")
            rs_out = dram.tile([L // 4, D], BF16, tag="rs_out",
                               name="rs_out")
            bc_dram = dram.tile([2 * DS, L], BF16, tag="bc_dram",
                                name="bc_dram")

            # packed constants: [0:32 convw(8g x 4)][32:40 convb]
            # [40:44 b_dt][44:108 A][108:112 dskip][112:113 eps]
            cst = per.tile([128, 128], F32, tag="cst", name="cst")
            nc.sync.dma_start(cst[:], consts_d.ap())
            convw = cst[:, 0:32]
            convb = cst[:, 32:40]
            b_dt_sb = cst[:, 40:44]
            A_sb = cst[:, 44:108]
            dskip_sb = cst[:, 108:112]
            eps_sb = cst[:, 112:113]
            sidx_sb = per.tile([128, NT], I32, tag="sidx", name="sidx")
            nc.sync.dma_start(sidx_sb[:], sidx_d.ap())
            ident = per.tile([128, 128], BF16, tag="ident", name="ident")
            nc.sync.dma_start(ident[:], ident_d.ap())
            wout_sb = per.tile([128, NG * D], BF16, tag="wout",
                               name="wout")
            nc.sync.dma_start(wout_sb[:], wout_d.ap())

            # persistent activations
            dt = [per.tile([128, L], BF16, tag=f"dt{g}", name=f"dt{g}")
                  for g in range(NG)]
            du = [per.tile([128, L], BF16, tag=f"du{g}", name=f"du{g}")
                  for g in range(NG)]
            # sk holds the D_skip*u term (fold seed); the fold result is
            # evicted back over it per half
            sk = [per.tile([128, L], BF16, tag=f"sk{g}", name=f"sk{g}")
                  for g in range(NG)]
            z = [per.tile([128, L], BF16, tag=f"z{g}", name=f"z{g}")
                 for g in range(NG)]
            y_mm = [per.tile([128, L], BF16, tag=f"ymm{g}",
                             name=f"ymm{g}") for g in range(NG)]
            carry = [per.tile([128, DS], F32, tag=f"carry{g}",
                              name=f"carry{g}") for g in range(NG)]

            # B/C broadcast staging: [128, QS*LH] quad tiles, two rotating
            # slots each (per state-quad parity) so loads prefetch two
            # quads ahead of the scan chain.  Loaded per (half, pass).
            # Created before the scoped A/C/D pools (pool stack is LIFO).
            bcp = ctx.enter_context(tc.tile_pool(name="bcp", bufs=1))

            def load_bc_np(h, np_):
                """Returns (bq, cq) tiles for state-quad np_ of half h and
                emits their broadcast loads."""
                hsl = slice(h * LH, (h + 1) * LH)
                bqt = bcp.tile([128, QS * LH], BF16, tag=f"bq{np_ % 2}",
                               name=f"bq{h}_{np_}")
                cqt = bcp.tile([128, QS * LH], BF16, tag=f"cq{np_ % 2}",
                               name=f"cq{h}_{np_}")
                # B on the SP hardware-DGE queue, C on the Pool queue: two
                # queues halve the serial broadcast latency per quad
                for qi in range(QS):
                    n = np_ * QS + qi
                    nc.sync.dma_start(
                        bqt[:, qi * LH:(qi + 1) * LH],
                        bc_dram[n:n + 1, hsl].to_broadcast([128, LH]))
                    nc.gpsimd.dma_start(
                        cqt[:, qi * LH:(qi + 1) * LH],
                        bc_dram[DS + n:DS + n + 1,
                                hsl].to_broadcast([128, LH]))
                return bqt, cqt

            # ---------------- Phase A: in_proj + conv + silu ----------
            adctx = ExitStack()
            phu = adctx.enter_context(tc.tile_pool(name="phu", bufs=1))
            u = [phu.tile([128, L], BF16, tag=f"u{m}", name=f"u{m}")
                 for m in range(NGF)]
            with ExitStack() as actx:
                pha = actx.enter_context(tc.tile_pool(name="pha", bufs=1))
                scr = actx.enter_context(tc.tile_pool(name="scr", bufs=2))
                psA = actx.enter_context(
                    tc.tile_pool(name="psA", bufs=2, space="PSUM"))
                xpad_sb = [pha.tile([128, DCONV - 1 + L], BF16,
                                    tag=f"xp{k}", name=f"xp{k}")
                           for k in range(4)]
                winT_sb = [pha.tile([128, DI], BF16, tag=f"wi{k}",
                                    name=f"wi{k}") for k in range(4)]
                wz_sb = [pha.tile([128, DH], BF16, tag=f"wzk{k}",
                                  name=f"wzk{k}") for k in range(4)]
                for k in range(4):
                    sl = slice(k * 128, (k + 1) * 128)
                    nc.sync.dma_start(xpad_sb[k][:], xpad_d.ap()[sl, :])
                    nc.sync.dma_start(winT_sb[k][:], winT_d.ap()[sl, :])
                    nc.sync.dma_start(wz_sb[k][:], wz_d.ap()[sl, :])

                # Per-chunk pipeline (nn-outer): in_proj + conv for all 8
                # groups of a 512-token chunk, then that chunk's xproj and
                # (on odd chunks) the dt/du chain — so the proj spills, B/C
                # broadcasts and dt all land long before phase E needs them.
                psC = actx.enter_context(
                    tc.tile_pool(name="psC", bufs=2, space="PSUM"))
                psD = actx.enter_context(
                    tc.tile_pool(name="psD", bufs=2, space="PSUM"))
                phd = actx.enter_context(tc.tile_pool(name="phd", bufs=2))
                xccp = actx.enter_context(tc.tile_pool(name="xccp",
                                                       bufs=10))
                wx_sb = pha.tile([128, NGF * NPROJ], BF16, tag="wx",
                                 name="wx")
                nc.sync.dma_start(wx_sb[:], wx_d.ap())
                wdt_sb = pha.tile([DTR, DH], BF16, tag="wdt", name="wdt")
                nc.sync.dma_start(wdt_sb[:], wdt_d.ap())
                projT = pha.tile([NPROJ, L], BF16, tag="projT",
                                 name="projT")
                prev_xcc = {}
                for nn in range(NC512):
                    for m in range(NGF):
                        ps = psA.tile([128, 512], F32, tag="mmA", name="mmA")
                        for k in range(4):
                            nc.tensor.matmul(
                                out=ps[:],
                                lhsT=winT_sb[k][:, m * 128:(m + 1) * 128],
                                rhs=xpad_sb[k][:, DCONV - 1 + nn * 512:
                                               DCONV - 1 + (nn + 1) * 512],
                                start=(k == 0), stop=(k == 3))
                        # conv per 512-chunk with a 3-column overlap staged
                        # into each chunk tile (col k of xcc is
                        # xc[nn*512 + k - 3]); kills the 8 full-L xc tiles
                        xcc = xccp.tile([128, 515], BF16, tag="xcc",
                                        name="xcc")
                        nc.scalar.copy(xcc[:, 3:515], ps[:])
                        if nn == 0:
                            nc.vector.memset(xcc[:, 0:3], 0.0)
                        else:
                            nc.vector.tensor_copy(xcc[:, 0:3],
                                                  prev_xcc[m][:, 512:515])
                        prev_xcc[m] = xcc
                        # causal conv: u[t] = silu(b + sum_j w[3-j]*xc[t-j])
                        a_t = scr.tile([128, 512], BF16, tag="cacc",
                                       name="cacc")
                        nc.vector.tensor_scalar(
                            a_t[:], xcc[:, 3:515],
                            convw[:, m * DCONV + DCONV - 1:
                                  m * DCONV + DCONV],
                            convb[:, m:m + 1], OP.mult, OP.add)
                        for j in range(1, DCONV):
                            nc.vector.scalar_tensor_tensor(
                                a_t[:], xcc[:, 3 - j:515 - j],
                                convw[:, m * DCONV + DCONV - 1 - j:
                                      m * DCONV + DCONV - j],
                                a_t[:], OP.mult, OP.add)
                        nc.scalar.activation(
                            u[m][:, nn * 512:(nn + 1) * 512], a_t[:],
                            AF.Silu)
                    ps = psC.tile([NPROJ, 512], F32, tag="mmC", name="mmC")
                    for kg in range(NGF):
                        nc.tensor.matmul(
                            out=ps[:],
                            lhsT=wx_sb[:, kg * NPROJ:(kg + 1) * NPROJ],
                            rhs=u[kg][:, nn * 512:(nn + 1) * 512],
                            start=(kg == 0), stop=(kg == NGF - 1))
                    nc.scalar.copy(projT[:, nn * 512:(nn + 1) * 512], ps[:])
                    nc.sync.dma_start(
                        bc_dram[:, nn * 512:(nn + 1) * 512],
                        projT[DTR:NPROJ, nn * 512:(nn + 1) * 512])
                    if nn == 1:
                        # first half of bc_dram is complete: start the
                        # broadcast loads for the scan's first two state
                        # quads now so the DVE never waits at phase-E entry
                        bc_pre = {np_: load_bc_np(0, np_)
                                  for np_ in range(2)}
                    if nn % 2 == 1:
                        hh = nn // 2
                        for g in range(NG):
                            ps2 = psD.tile([128, 1024], F32, tag="mmD",
                                           name="mmD")
                            for ci in range(2):
                                c = nn - 1 + ci
                                nc.tensor.matmul(
                                    out=ps2[:, ci * 512:(ci + 1) * 512],
                                    lhsT=wdt_sb[:, g * 128:(g + 1) * 128],
                                    rhs=projT[0:DTR,
                                              c * 512:(c + 1) * 512],
                                    start=True, stop=True)
                            edt = phd.tile([128, 1024], F32, tag="edt",
                                           name="edt")
                            nc.scalar.activation(edt[:], ps2[:], AF.Exp,
                                                 bias=b_dt_sb[:, g:g + 1])
                            nc.scalar.activation(
                                dt[g][:, hh * LH:(hh + 1) * LH], edt[:],
                                AF.Ln, bias=1.0)
                        # du/sk per half right after its dt lands
                        for g in range(NG):
                            hs = slice(hh * LH, (hh + 1) * LH)
                            nc.vector.tensor_tensor(out=du[g][:, hs],
                                                    in0=dt[g][:, hs],
                                                    in1=u[g][:, hs],
                                                    op=OP.mult)
                            nc.vector.tensor_scalar(
                                sk[g][:, hs], u[g][:, hs],
                                dskip_sb[:, g:g + 1], None, OP.mult)
                # z (own half) -> silu -> z[g]; only needed by the
                # pass-end gates, so emitted after the dt/du chain
                for g in range(NG):
                    for nn in range(NC512):
                        ps = psA.tile([128, 512], F32, tag="mmA", name="mmA")
                        for k in range(4):
                            nc.tensor.matmul(
                                out=ps[:],
                                lhsT=wz_sb[k][:, g * 128:(g + 1) * 128],
                                rhs=xpad_sb[k][:, DCONV - 1 + nn * 512:
                                               DCONV - 1 + (nn + 1) * 512],
                                start=(k == 0), stop=(k == 3))
                        nc.scalar.activation(
                            z[g][:, nn * 512:(nn + 1) * 512], ps[:],
                            AF.Silu)
            adctx.close()  # frees the u tiles before phase E

            # ---------------- Phase E: selective scan ----------------
            with ExitStack() as ectx:
                trb = ectx.enter_context(tc.tile_pool(name="trb", bufs=1))
                # dA double-buffered so the Act exps prefetch a full
                # (g, state-quad) iteration ahead of the DVE scan chain
                dApool = ectx.enter_context(tc.tile_pool(name="dAp",
                                                         bufs=2))
                psY = ectx.enter_context(
                    tc.tile_pool(name="psY", bufs=1, space="PSUM"))
                psF = ectx.enter_context(
                    tc.tile_pool(name="psF", bufs=4, space="PSUM"))
                osb_pool = ectx.enter_context(tc.tile_pool(name="osb",
                                                           bufs=4))

                first_pass_pre = bc_pre
                for h in range(NH):
                    hsl = slice(h * LH, (h + 1) * LH)
                    for gp in range(NG // 2):
                        gs = [2 * gp, 2 * gp + 1]
                        # B/C quad tiles for this pass: the first two come
                        # either from the C/D-phase prefetch (very first
                        # pass) or are loaded now; later quads are loaded
                        # two ahead of the scan chain inside the np loop.
                        bct = first_pass_pre if first_pass_pre else {
                            np_: load_bc_np(h, np_) for np_ in range(2)}
                        first_pass_pre = None
                        ytiles = {g: [psY.tile([128, 512], F32,
                                               tag=f"psY{g % 2}_{q}",
                                               name=f"psY{g}_{h}_{q}")
                                      for q in range(2)] for g in gs}
                        # seed the fold chains with the D_skip*u term
                        for g in gs:
                            for q in range(2):
                                nc.tensor.matmul(
                                    out=ytiles[g][q][:],
                                    lhsT=ident[:],
                                    rhs=sk[g][:, h * LH + q * 512:
                                              h * LH + (q + 1) * 512],
                                    start=True, stop=False)

                        def w_mult(dst, g, bqt, eng):
                            eng.tensor_tensor(
                                out=dst[:].rearrange(
                                    "p (q l) -> p q l", q=QS),
                                in0=du[g][:, hsl].rearrange(
                                    "p (q l) -> p q l",
                                    q=1).to_broadcast([128, QS, LH]),
                                in1=bqt[:].rearrange(
                                    "p (q l) -> p q l", q=QS),
                                op=OP.mult)

                        for np_ in range(NQ):
                            if np_ + 2 < NQ:
                                bct[np_ + 2] = load_bc_np(h, np_ + 2)
                            for g in gs:
                                dA = dApool.tile([128, QS * LH], BF16,
                                                 tag="dAp", name="dA")
                                for qi in range(QS):
                                    n = np_ * QS + qi
                                    nidx = g * DS + n
                                    nc.scalar.activation(
                                        dA[:, qi * LH:(qi + 1) * LH],
                                        dt[g][:, hsl], AF.Exp,
                                        scale=A_sb[:, nidx:nidx + 1])
                                w_t = trb.tile([128, QS * LH], BF16,
                                               tag="w", name="w")
                                w_mult(w_t, g, bct[np_][0], nc.vector)
                                h_t = trb.tile([128, QS * LH], BF16,
                                               tag="h", name="h")
                                for qi in range(QS):
                                    n = np_ * QS + qi
                                    init = (0.0 if h == 0
                                            else carry[g][:, n:n + 1])
                                    nc.vector.tensor_tensor_scan(
                                        h_t[:, qi * LH:(qi + 1) * LH],
                                        dA[:, qi * LH:(qi + 1) * LH],
                                        w_t[:, qi * LH:(qi + 1) * LH],
                                        init, OP.mult, OP.add)
                                if h == 0:
                                    # save the carried state (last column
                                    # of each state's chunk)
                                    nc.scalar.copy(
                                        carry[g][:, np_ * QS:
                                                 (np_ + 1) * QS].rearrange(
                                            "p (q o) -> p q o", o=1),
                                        h_t[:].rearrange(
                                            "p (q l) -> p q l",
                                            q=QS)[:, :, LH - 1:LH])
                                p_t = trb.tile([128, QS * LH], BF16,
                                               tag="p", name="p")
                                nc.vector.tensor_tensor(
                                    out=p_t[:], in0=h_t[:],
                                    in1=bct[np_][1][:],
                                    op=OP.mult)
                                last = (np_ == NQ - 1)
                                for qi in range(QS):
                                    for q in range(2):
                                        nc.tensor.matmul(
                                            out=ytiles[g][q][:],
                                            lhsT=ident[:],
                                            rhs=p_t[:, qi * LH + q * 512:
                                                    qi * LH + (q + 1) * 512],
                                            start=False,
                                            stop=(last and qi == QS - 1))
                        # eviction + silu(z) gate for this pass's 2 groups
                        for g in gs:
                            for q in range(2):
                                nc.scalar.copy(
                                    sk[g][:, h * LH + q * 512:
                                          h * LH + (q + 1) * 512],
                                    ytiles[g][q][:])
                            nc.vector.tensor_tensor(
                                out=y_mm[g][:, hsl], in0=sk[g][:, hsl],
                                in1=z[g][:, hsl], op=OP.mult)

                    # ---- Phase F for this half: out_proj + scatter ----
                    for j in range(L // NH // 128):
                        tt = h * (LH // 128) + j
                        ps = psF.tile([128, D], F32, tag="mmF", name="mmF")
                        for g in range(NG):
                            nc.tensor.matmul(
                                out=ps[:],
                                lhsT=y_mm[g][:, tt * 128:(tt + 1) * 128],
                                rhs=wout_sb[:, g * D:(g + 1) * D],
                                start=(g == 0), stop=(g == NG - 1))
                        o_sb = osb_pool.tile([128, D], BF16, tag="osb",
                                             name="osb")
                        nc.scalar.copy(o_sb[:], ps[:])
                        nc.gpsimd.indirect_dma_start(
                            out=out_bounce.opt(),
                            out_offset=bass.IndirectOffsetOnAxis(
                                ap=sidx_sb[:, tt:tt + 1], axis=0),
                            in_=o_sb[:],
                            in_offset=None)

            # -------- Phase G: ReduceScatter + LN/SiLU/residual --------
            with ExitStack() as gctx:
                phg2 = gctx.enter_context(tc.tile_pool(name="phg2",
                                                       bufs=1))
                # prefetch the residual/LN params before the collective so
                # their DMAs overlap the ReduceScatter wait
                xres_sb = phg2.tile([128, 4 * D], F32, tag="xres",
                                    name="xres")
                for qq in range(4):
                    nc.sync.dma_start(xres_sb[:, qq * D:(qq + 1) * D],
                                      xres_d.ap()[qq * 128:(qq + 1) * 128,
                                                  :])
                lng_sb = phg2.tile([128, D], BF16, tag="lng", name="lng")
                nc.sync.dma_start(lng_sb[:], ln_g_d.ap())
                lnb_sb = phg2.tile([128, D], BF16, tag="lnb", name="lnb")
                nc.sync.dma_start(lnb_sb[:], ln_b_d.ap())
                if for_timeline:
                    nc.sync.dma_start(rs_out[:], out_bounce[0:L // 4, :])
                else:
                    nc.gpsimd.collective_compute(
                        "ReduceScatter", OP.add, replica_groups=quad_groups,
                        ins=[out_bounce.opt()], outs=[rs_out.opt()])
                rs_bf = phg2.tile([128, 4 * D], BF16, tag="rsb",
                                  name="rsb")
                for qq in range(4):
                    nc.sync.dma_start(rs_bf[:, qq * D:(qq + 1) * D],
                                      rs_out[qq * 128:(qq + 1) * 128, :])

                for qq in range(4):
                    rtile = rs_bf[:, qq * D:(qq + 1) * D]
                    st = phg2.tile([128, 8], F32, tag=f"st{qq}",
                                   name=f"st{qq}")
                    nc.vector.bn_stats(st[:, 0:6], rtile)
                    nc.vector.bn_aggr(st[:, 6:8], st[:, 0:6])
                    mu = st[:, 6:7]
                    var = st[:, 7:8]
                    std = phg2.tile([128, 1], F32, tag=f"std{qq}",
                                    name=f"std{qq}")
                    nc.scalar.activation(std[:], var[:], AF.Sqrt,
                                         bias=eps_sb[:, 0:1])
                    rstd = phg2.tile([128, 1], F32, tag=f"rstd{qq}",
                                     name=f"rstd{qq}")
                    nc.vector.reciprocal(rstd[:], std[:])
                    nmr = phg2.tile([128, 1], F32, tag=f"nmr{qq}",
                                    name=f"nmr{qq}")
                    nc.vector.tensor_scalar(nmr[:], mu[:], rstd[:, 0:1],
                                            -1.0, OP.mult, OP.mult)
                    s1 = phg2.tile([128, D], BF16, tag=f"s1{qq}",
                                   name=f"s1{qq}")
                    nc.scalar.activation(s1[:], rtile, AF.Identity,
                                         scale=rstd[:, 0:1],
                                         bias=nmr[:, 0:1])
                    s2 = phg2.tile([128, D], BF16, tag=f"s2{qq}",
                                   name=f"s2{qq}")
                    nc.vector.tensor_tensor(out=s2[:], in0=s1[:],
                                            in1=lng_sb[:], op=OP.mult)
                    s3 = phg2.tile([128, D], BF16, tag=f"s3{qq}",
                                   name=f"s3{qq}")
                    nc.vector.tensor_tensor(out=s3[:], in0=s2[:],
                                            in1=lnb_sb[:], op=OP.add)
                    sil = phg2.tile([128, D], F32, tag=f"sil{qq}",
                                    name=f"sil{qq}")
                    nc.scalar.activation(sil[:], s3[:], AF.Silu)
                    fin = phg2.tile([128, D], F32, tag=f"fin{qq}",
                                    name=f"fin{qq}")
                    nc.vector.tensor_tensor(
                        out=fin[:], in0=sil[:],
                        in1=xres_sb[:, qq * D:(qq + 1) * D], op=OP.add)
                    nc.sync.dma_start(
                        out_d.ap()[qq * 128:(qq + 1) * 128, :], fin[:])

    _legalize_waits(nc)
    return nc


_NC_CACHE = {}


def _get_nc():
    if "nc" not in _NC_CACHE:
        _NC_CACHE["nc"] = _build_nc()
    return _NC_CACHE["nc"]


def _pg(a, ngroups):
    """[ngroups*128, k] -> [128, ngroups*k] partition-major regroup."""
    k = a.shape[1]
    return np.ascontiguousarray(
        a.reshape(ngroups, 128, k).transpose(1, 0, 2)).reshape(
            128, ngroups * k)


def _prep_in_maps(inputs):
    import ml_dtypes
    BF = ml_dtypes.bfloat16
    x = np.asarray(inputs["x"], np.float32)
    ln_g = np.asarray(inputs["ln_g"], np.float32)
    ln_b = np.asarray(inputs["ln_b"], np.float32)
    ln_gb = np.broadcast_to(ln_g[None, :], (128, D)).copy()
    ln_bb = np.broadcast_to(ln_b[None, :], (128, D)).copy()

    in_maps = []
    for c in range(NCORE):
        b, dr, half = c // 4, (c % 4) // 2, c % 2
        pfx = "fw_" if dr == 0 else "bw_"
        W_in = np.asarray(inputs[pfx + "W_in"], np.float32)
        convw = np.asarray(inputs[pfx + "conv_w"], np.float32)
        convb = np.asarray(inputs[pfx + "conv_b"], np.float32)
        W_x = np.asarray(inputs[pfx + "W_xproj"], np.float32)
        W_dt = np.asarray(inputs[pfx + "W_dt"], np.float32)
        b_dt = np.asarray(inputs[pfx + "b_dt"], np.float32)
        A_log = np.asarray(inputs[pfx + "A_log"], np.float32)
        dskip = np.asarray(inputs[pfx + "D_skip"], np.float32)
        W_out = np.asarray(inputs[pfx + "W_out"], np.float32)

        h0 = half * DH
        oh0 = (1 - half) * DH
        perm = np.concatenate([np.arange(h0, h0 + DH),
                               np.arange(oh0, oh0 + DH)])
        own = perm[:DH]

        xb = x[b] if dr == 0 else x[b][::-1]
        xpad = np.zeros((D, DCONV - 1 + L), np.float32)
        xpad[:, DCONV - 1:] = xb.T

        q = c % 4
        t = np.arange(NT * 128).reshape(NT, 128)
        rows = t if dr == 0 else (L - 1) - t

        consts = np.zeros((128, 128), np.float32)
        consts[:, 0:32] = _pg(convw[perm], NGF)
        consts[:, 32:40] = _pg(convb[perm][:, None], NGF)
        consts[:, 40:44] = _pg(b_dt[own][:, None], NG)
        consts[:, 44:108] = _pg(-np.exp(A_log[own]), NG)
        consts[:, 108:112] = _pg(dskip[own][:, None], NG)
        consts[:, 112] = 1e-5

        wx = np.empty((128, NGF * NPROJ), np.float32)
        for kg in range(NGF):
            ch = perm[kg * 128:(kg + 1) * 128]
            wx[:, kg * NPROJ:(kg + 1) * NPROJ] = W_x[:, ch].T

        m = {
            "xpad": xpad.astype(BF),
            "winT": np.ascontiguousarray(
                W_in[:DI][perm].T).astype(BF),
            "wz": np.ascontiguousarray(
                W_in[DI + h0:DI + h0 + DH].T).astype(BF),
            "wx": wx.astype(BF),
            "wdt": np.ascontiguousarray(W_dt[own].T).astype(BF),
            "wout": _pg(0.5 * W_out[:, own].T, NG).astype(BF),
            "consts": consts,
            "xres": np.ascontiguousarray(
                x[b][q * (L // 4):(q + 1) * (L // 4)]),
            "ln_g": ln_gb.astype(BF),
            "ln_b": ln_bb.astype(BF),
            "sidx": np.ascontiguousarray(rows.T.astype(np.int32)),
            "ident": np.eye(128, dtype=np.float32).astype(BF),
        }
        in_maps.append(m)
    return in_maps


def _assemble(results):
    out = np.empty((B, L, D), np.float32)
    for b in range(B):
        out[b] = np.concatenate(
            [results[4 * b + q]["out_shard"] for q in range(4)], axis=0)
    return out


def _run(inputs, trace=False, **kw):
    nc = _get_nc()
    in_maps = _prep_in_maps(inputs)
    res = bass_utils.run_bass_kernel_spmd(
        nc, in_maps, core_ids=list(range(NCORE)), trace=trace, **kw)
    return _assemble(res.results), res


def _timed_run(inputs, iters=10):
    """Run once for outputs, then time repeated executions of the jitted
    sharded body (no donation; inputs resident on device)."""
    import jax
    import numpy as _np
    from jax.sharding import Mesh, PartitionSpec, NamedSharding
    from jax.experimental.shard_map import shard_map
    import concourse.bass2jax as bass2jax
    import concourse.mybir as _mybir

    nc = _get_nc()
    in_maps = _prep_in_maps(inputs)
    bass2jax.install_neuronx_cc_hook()

    partition_name = (nc.partition_id_tensor.name
                      if nc.partition_id_tensor else None)
    in_names, out_names, out_avals, zero_outs = [], [], [], []
    for alloc in nc.m.functions[0].allocations:
        if not isinstance(alloc, _mybir.MemoryLocationSet):
            continue
        name = alloc.memorylocations[0].name
        if alloc.kind == "ExternalInput":
            if name != partition_name:
                in_names.append(name)
        elif alloc.kind == "ExternalOutput":
            shape = tuple(alloc.tensor_shape)
            dtype = _mybir.dt.np(alloc.dtype)
            out_names.append(name)
            out_avals.append(jax.core.ShapedArray(shape, dtype))
            zero_outs.append(_np.zeros(shape, dtype))
    n_params = len(in_names)
    n_outs = len(out_avals)
    all_in_names = list(in_names) + list(out_names)
    if partition_name is not None:
        all_in_names.append(partition_name)

    def _body(*args):
        operands = list(args)
        if partition_name is not None:
            operands.append(bass2jax.partition_id_tensor())
        outs = bass2jax._bass_exec_p.bind(
            *operands,
            out_avals=tuple(out_avals),
            in_names=tuple(all_in_names),
            out_names=tuple(out_names),
            lowering_input_output_aliases=(),
            sim_require_finite=True,
            sim_require_nnan=True,
            nc=nc,
        )
        return tuple(outs)

    devices = jax.devices()[:NCORE]
    mesh = Mesh(_np.asarray(devices), ("core",))
    in_specs = (PartitionSpec("core"),) * (n_params + n_outs)
    out_specs = (PartitionSpec("core"),) * n_outs
    donate = tuple(range(n_params, n_params + n_outs))
    sharded = jax.jit(
        shard_map(_body, mesh=mesh, in_specs=in_specs, out_specs=out_specs,
                  check_rep=False),
        donate_argnums=donate, keep_unused=True)

    sh = NamedSharding(mesh, PartitionSpec("core"))
    concat_in = [
        jax.device_put(_np.concatenate(
            [_np.asarray(in_maps[c][nm]) for c in range(NCORE)], axis=0), sh)
        for nm in in_names
    ]
    def make_zeros():
        return [jax.device_put(
            _np.zeros((NCORE * z.shape[0], *z.shape[1:]), z.dtype), sh)
            for z in zero_outs]

    jax.block_until_ready(concat_in)
    z0 = make_zeros()
    jax.block_until_ready(z0)
    out_arrs = jax.block_until_ready(sharded(*concat_in, *z0))
    results = [
        {nm: _np.asarray(out_arrs[i]).reshape(NCORE, *out_avals[i].shape)[c]
         for i, nm in enumerate(out_names)}
        for c in range(NCORE)
    ]
    output = _assemble(results)

    zsets = [make_zeros() for _ in range(iters)]
    jax.block_until_ready(zsets)
    best = None
    for zi in zsets:
        t1 = time.perf_counter()
        jax.block_until_ready(sharded(*concat_in, *zi))
        dt_s = time.perf_counter() - t1
        best = dt_s if best is None else min(best, dt_s)
    return output, int(best * 1e9)


def kernel(**inputs):
    out, _ = _run(inputs)
    return out


# revision 40
# speedup vs baseline: 1.2405x; 1.0081x over previous
# Bidirectional Mamba block on 8 TRN2 NeuronCores — v7 (~684 us/core HW,
# vs ~702 us for v2; phase E runs with zero DVE bubbles).
#
# Sharding: core c = (b, dir, half): b = c // 4, dir = (c % 4) // 2,
# half = c % 2.  Each core runs one direction of one batch element for half
# (512) of the d_inner channels, computing the in_proj/conv/silu (u) for
# ALL 1024 channels so the x-projection needs no cross-core AllReduce.
# The only collective is the final 4-way ReduceScatter of the output
# projection partials.
#
# Structure (changes vs v2):
#  - head is one per-512-token-chunk pipeline: in_proj matmuls + causal
#    conv (3-col-overlap chunk tiles, no full-L xc staging) + x-projection
#    + dt=softplus (merged [128,1024] Exp/Ln) per chunk, so the proj
#    spills, B/C broadcasts and dt land long before phase E needs them;
#    the z projection (only needed by the pass-end gates) comes last.
#  - selective scan split into two L/2 chunks; the recurrent state crosses
#    the boundary via the scan's per-partition AP initial (per-state
#    scans, no Q-merge).  Phase F (out_proj) + fold eviction of the first
#    half run on PE/Act during the second half's scans.
#  - B/C broadcast quads [128, 4*1024] stream just-in-time two quads ahead
#    of the scan chain, B on the SP DMA queue / C on the Pool queue (a
#    single queue head-of-line-blocks).
#  - dA exp needs no memset/shift: chunk 0 uses initial=0.0 (dA[0] is
#    multiplied by 0), chunk 1 the carried state with dA = exp(A*dt).
#  - z gate kept resident in SBUF (no DRAM spill round-trip).
#  - LayerNorm via bn_stats/bn_aggr + fused scale-bias activation.
# Measured per-op costs and engine pitfalls (Pool/DVE SBUF port sharing,
# in-place TT pathology, STT stuck at 1x) are in the session notes.
import time
import numpy as np
from contextlib import ExitStack

import concourse.bass as bass
import concourse.mybir as mybir
import concourse.tile as tile
from concourse import bass_utils

F32 = mybir.dt.float32
BF16 = mybir.dt.bfloat16
I32 = mybir.dt.int32
AF = mybir.ActivationFunctionType
OP = mybir.AluOpType

B, L, D = 2, 2048, 512
DI, DS, DTR, DCONV = 1024, 16, 32, 4
NCORE = 8
DH = DI // 2            # d_inner channels per core (own half)
NGF = DI // 128         # 8 channel groups of 128 (full)
NG = DH // 128          # 4 own channel groups
NT = L // 128           # 16 token tiles
NC512 = L // 512        # 4 chunks of 512 along t

NH = 2                  # L-halves for the chunked scan
LH = L // NH            # 1024 tokens per half
QS = 4                  # states per quad tile
NQ = DS // QS           # 4 state quads
NPROJ = DTR + 2 * DS
# NOTE: offloading elementwise work to the Pool engine was measured to be
# a net loss — Pool TTs are ~7x slower than DVE *and* share SBUF ports
# with it, slowing concurrent DVE scans/TTs by 2-4x.


def _legalize_waits(nc, max_waits=1):
    """walrus's per-instruction sync-wait slots are limited (a Matmult with 2
    waits fails codegen).  Move excess waits onto a same-engine
    InstEventSemaphore inserted right before the instruction."""
    skip = ("InstEventSemaphore", "InstBassTrap",
            "InstTriggeredCopy", "InstNoOp",
            "InstDMAGatherAnt", "InstDMAScatterAddAnt", "InstTensorLoad",
            "InstTensorSave", "InstRegisterMove", "InstUnconditionalBranch")
    eng_map = {
        mybir.EngineType.DVE: nc.vector,
        mybir.EngineType.Activation: nc.scalar,
        mybir.EngineType.PE: nc.tensor,
        mybir.EngineType.Pool: nc.gpsimd,
        mybir.EngineType.SP: nc.sync,
    }
    n_split = 0
    for fn in nc.m.functions:
        for bb in fn.blocks:
            for target in list(bb.instructions):
                si = target.sync_info
                tname = type(target).__name__
                if (si is None or not si.on_wait
                        or len(si.on_wait) <= max_waits or tname in skip):
                    continue
                excess = list(si.on_wait[:-max_waits])
                keep = list(si.on_wait[-max_waits:])
                si.on_wait = keep
                # chain EventSemaphores, each carrying <= 2 waits
                for i0 in range(0, len(excess), 2):
                    ev = mybir.InstEventSemaphore(
                        name=nc.get_next_instruction_name(),
                        ins=[], outs=[],
                        sync_info=mybir.SyncInfo(
                            on_wait=excess[i0:i0 + 2], on_update=[]))
                    eng_map[target.engine].add_instruction(ev)
                    tail_bb = nc.m.functions[-1].blocks[-1]
                    evi = tail_bb.instructions[-1]
                    assert evi.name == ev.name
                    tail_insts = list(tail_bb.instructions)
                    tail_insts.pop()
                    tail_bb.instructions = tail_insts
                    insts = list(bb.instructions)
                    insts.insert(insts.index(target), evi)
                    bb.instructions = insts
                n_split += 1
    return n_split


def _build_nc(for_timeline=False):
    nc = bass.Bass("TRN2", target_bir_lowering=False, debug=False,
                   num_devices=NCORE)

    # ---------------- I/O declarations (per core) ----------------
    xpad_d = nc.dram_tensor("xpad", [D, DCONV - 1 + L], BF16,
                            kind="ExternalInput")
    winT_d = nc.dram_tensor("winT", [D, DI], BF16, kind="ExternalInput")
    wz_d = nc.dram_tensor("wz", [D, DH], BF16, kind="ExternalInput")
    wx_d = nc.dram_tensor("wx", [128, NGF * NPROJ], BF16,
                          kind="ExternalInput")
    wdt_d = nc.dram_tensor("wdt", [DTR, DH], BF16, kind="ExternalInput")
    wout_d = nc.dram_tensor("wout", [128, NG * D], BF16,
                            kind="ExternalInput")
    consts_d = nc.dram_tensor("consts", [128, 128], F32,
                              kind="ExternalInput")
    xres_d = nc.dram_tensor("xres", [L // 4, D], F32, kind="ExternalInput")
    ln_g_d = nc.dram_tensor("ln_g", [128, D], BF16, kind="ExternalInput")
    ln_b_d = nc.dram_tensor("ln_b", [128, D], BF16, kind="ExternalInput")
    sidx_d = nc.dram_tensor("sidx", [128, NT], I32, kind="ExternalInput")
    ident_d = nc.dram_tensor("ident", [128, 128], BF16,
                             kind="ExternalInput")
    out_d = nc.dram_tensor("out_shard", [L // 4, D], F32,
                           kind="ExternalOutput")

    # NOTE: the 4-way "Mesh" RS is rank-asymmetric on HW (ranks 0-1 take
    # 52-73us, ranks 2-3 only 33-37us), but splitting it into two
    # pairwise ReduceScatter rounds measured ~150us SLOWER overall —
    # per-collective fixed overhead dominates.  Keep the single 4-way op.
    quad_groups = [[0, 1, 2, 3], [4, 5, 6, 7]]

    with tile.TileContext(nc) as tc:
        with ExitStack() as ctx:
            per = ctx.enter_context(tc.tile_pool(name="per", bufs=1))
            dram = ctx.enter_context(tc.tile_pool(name="dram", bufs=1,
                                                  space="DRAM"))

            out_bounce = dram.tile([L, D], BF16, tag="out_bounce",
                                   name="out_bounce")
            rs_out = dram.tile([L // 4, D], BF16, tag="rs_out",
                               name="rs_out")
            bc_dram = dram.tile([2 * DS, L], BF16, tag="bc_dram",
                                name="bc_dram")

            # packed constants: [0:32 convw(8g x 4)][32:40 convb]
            # [40:44 b_dt][44:108 A][108:112 dskip][112:113 eps]
            cst = per.tile([128, 128], F32, tag="cst", name="cst")
            nc.sync.dma_start(cst[:], consts_d.ap())
            convw = cst[:, 0:32]
            convb = cst[:, 32:40]
            b_dt_sb = cst[:, 40:44]
            A_sb = cst[:, 44:108]
            dskip_sb = cst[:, 108:112]
            eps_sb = cst[:, 112:113]
            sidx_sb = per.tile([128, NT], I32, tag="sidx", name="sidx")
            ident = per.tile([128, 128], BF16, tag="ident", name="ident")
            wout_sb = per.tile([128, NG * D], BF16, tag="wout",
                               name="wout")

            # persistent activations
            dt = [per.tile([128, L], BF16, tag=f"dt{g}", name=f"dt{g}")
                  for g in range(NG)]
            du = [per.tile([128, L], BF16, tag=f"du{g}", name=f"du{g}")
                  for g in range(NG)]
            # sk holds the D_skip*u term (fold seed); the fold result is
            # evicted back over it per half
            sk = [per.tile([128, L], BF16, tag=f"sk{g}", name=f"sk{g}")
                  for g in range(NG)]
            z = [per.tile([128, L], BF16, tag=f"z{g}", name=f"z{g}")
                 for g in range(NG)]
            y_mm = [per.tile([128, L], BF16, tag=f"ymm{g}",
                             name=f"ymm{g}") for g in range(NG)]
            carry = [per.tile([128, DS], F32, tag=f"carry{g}",
                              name=f"carry{g}") for g in range(NG)]

            # B/C broadcast staging: [128, QS*LH] quad tiles, two rotating
            # slots each (per state-quad parity) so loads prefetch two
            # quads ahead of the scan chain.  Loaded per (half, pass).
            # Created before the scoped A/C/D pools (pool stack is LIFO).
            bcp = ctx.enter_context(tc.tile_pool(name="bcp", bufs=1))

            def load_bc_np(h, np_):
                """Returns (bq, cq) tiles for state-quad np_ of half h and
                emits their broadcast loads."""
                hsl = slice(h * LH, (h + 1) * LH)
                bqt = bcp.tile([128, QS * LH], BF16, tag=f"bq{np_ % 2}",
                               name=f"bq{h}_{np_}")
                cqt = bcp.tile([128, QS * LH], BF16, tag=f"cq{np_ % 2}",
                               name=f"cq{h}_{np_}")
                # B on the SP hardware-DGE queue, C on the Pool queue: two
                # queues halve the serial broadcast latency per quad
                for qi in range(QS):
                    n = np_ * QS + qi
                    nc.sync.dma_start(
                        bqt[:, qi * LH:(qi + 1) * LH],
                        bc_dram[n:n + 1, hsl].to_broadcast([128, LH]))
                    nc.gpsimd.dma_start(
                        cqt[:, qi * LH:(qi + 1) * LH],
                        bc_dram[DS + n:DS + n + 1,
                                hsl].to_broadcast([128, LH]))
                return bqt, cqt

            # ---------------- Phase A: in_proj + conv + silu ----------
            adctx = ExitStack()
            phu = adctx.enter_context(tc.tile_pool(name="phu", bufs=1))
            u = [phu.tile([128, L], BF16, tag=f"u{m}", name=f"u{m}")
                 for m in range(NGF)]
            with ExitStack() as actx:
                pha = actx.enter_context(tc.tile_pool(name="pha", bufs=1))
                scr = actx.enter_context(tc.tile_pool(name="scr", bufs=2))
                psA = actx.enter_context(
                    tc.tile_pool(name="psA", bufs=2, space="PSUM"))
                xpad_sb = [pha.tile([128, DCONV - 1 + L], BF16,
                                    tag=f"xp{k}", name=f"xp{k}")
                           for k in range(4)]
                winT_sb = [pha.tile([128, DI], BF16, tag=f"wi{k}",
                                    name=f"wi{k}") for k in range(4)]
                wz_sb = [pha.tile([128, DH], BF16, tag=f"wzk{k}",
                                  name=f"wzk{k}") for k in range(4)]
                for k in range(4):
                    sl = slice(k * 128, (k + 1) * 128)
                    nc.sync.dma_start(xpad_sb[k][:], xpad_d.ap()[sl, :])
                    nc.sync.dma_start(winT_sb[k][:], winT_d.ap()[sl, :])
                    nc.sync.dma_start(wz_sb[k][:], wz_d.ap()[sl, :])
                # needed only by phases E/F — loaded after the in_proj
                # weights so they never delay the first matmul chain
                nc.sync.dma_start(sidx_sb[:], sidx_d.ap())
                nc.sync.dma_start(ident[:], ident_d.ap())
                nc.sync.dma_start(wout_sb[:], wout_d.ap())

                # Per-chunk pipeline (nn-outer): in_proj + conv for all 8
                # groups of a 512-token chunk, then that chunk's xproj and
                # (on odd chunks) the dt/du chain — so the proj spills, B/C
                # broadcasts and dt all land long before phase E needs them.
                psC = actx.enter_context(
                    tc.tile_pool(name="psC", bufs=2, space="PSUM"))
                psD = actx.enter_context(
                    tc.tile_pool(name="psD", bufs=2, space="PSUM"))
                phd = actx.enter_context(tc.tile_pool(name="phd", bufs=2))
                xccp = actx.enter_context(tc.tile_pool(name="xccp",
                                                       bufs=10))
                wx_sb = pha.tile([128, NGF * NPROJ], BF16, tag="wx",
                                 name="wx")
                nc.sync.dma_start(wx_sb[:], wx_d.ap())
                wdt_sb = pha.tile([DTR, DH], BF16, tag="wdt", name="wdt")
                nc.sync.dma_start(wdt_sb[:], wdt_d.ap())
                projT = pha.tile([NPROJ, L], BF16, tag="projT",
                                 name="projT")
                prev_xcc = {}
                for nn in range(NC512):
                    for m in range(NGF):
                        ps = psA.tile([128, 512], F32, tag="mmA", name="mmA")
                        for k in range(4):
                            nc.tensor.matmul(
                                out=ps[:],
                                lhsT=winT_sb[k][:, m * 128:(m + 1) * 128],
                                rhs=xpad_sb[k][:, DCONV - 1 + nn * 512:
                                               DCONV - 1 + (nn + 1) * 512],
                                start=(k == 0), stop=(k == 3))
                        # conv per 512-chunk with a 3-column overlap staged
                        # into each chunk tile (col k of xcc is
                        # xc[nn*512 + k - 3]); kills the 8 full-L xc tiles
                        xcc = xccp.tile([128, 515], BF16, tag="xcc",
                                        name="xcc")
                        nc.scalar.copy(xcc[:, 3:515], ps[:])
                        if nn == 0:
                            nc.vector.memset(xcc[:, 0:3], 0.0)
                        else:
                            nc.vector.tensor_copy(xcc[:, 0:3],
                                                  prev_xcc[m][:, 512:515])
                        prev_xcc[m] = xcc
                        # causal conv: u[t] = silu(b + sum_j w[3-j]*xc[t-j])
                        a_t = scr.tile([128, 512], BF16, tag="cacc",
                                       name="cacc")
                        nc.vector.tensor_scalar(
                            a_t[:], xcc[:, 3:515],
                            convw[:, m * DCONV + DCONV - 1:
                                  m * DCONV + DCONV],
                            convb[:, m:m + 1], OP.mult, OP.add)
                        for j in range(1, DCONV):
                            nc.vector.scalar_tensor_tensor(
                                a_t[:], xcc[:, 3 - j:515 - j],
                                convw[:, m * DCONV + DCONV - 1 - j:
                                      m * DCONV + DCONV - j],
                                a_t[:], OP.mult, OP.add)
                        nc.scalar.activation(
                            u[m][:, nn * 512:(nn + 1) * 512], a_t[:],
                            AF.Silu)
                    ps = psC.tile([NPROJ, 512], F32, tag="mmC", name="mmC")
                    for kg in range(NGF):
                        nc.tensor.matmul(
                            out=ps[:],
                            lhsT=wx_sb[:, kg * NPROJ:(kg + 1) * NPROJ],
                            rhs=u[kg][:, nn * 512:(nn + 1) * 512],
                            start=(kg == 0), stop=(kg == NGF - 1))
                    nc.scalar.copy(projT[:, nn * 512:(nn + 1) * 512], ps[:])
                    nc.sync.dma_start(
                        bc_dram[:, nn * 512:(nn + 1) * 512],
                        projT[DTR:NPROJ, nn * 512:(nn + 1) * 512])
                    if nn == 1:
                        # first half of bc_dram is complete: start the
                        # broadcast loads for the scan's first two state
                        # quads now so the DVE never waits at phase-E entry
                        bc_pre = {np_: load_bc_np(0, np_)
                                  for np_ in range(2)}
                    if nn % 2 == 1:
                        hh = nn // 2
                        for g in range(NG):
                            ps2 = psD.tile([128, 1024], F32, tag="mmD",
                                           name="mmD")
                            for ci in range(2):
                                c = nn - 1 + ci
                                nc.tensor.matmul(
                                    out=ps2[:, ci * 512:(ci + 1) * 512],
                                    lhsT=wdt_sb[:, g * 128:(g + 1) * 128],
                                    rhs=projT[0:DTR,
                                              c * 512:(c + 1) * 512],
                                    start=True, stop=True)
                            edt = phd.tile([128, 1024], F32, tag="edt",
                                           name="edt")
                            nc.scalar.activation(edt[:], ps2[:], AF.Exp,
                                                 bias=b_dt_sb[:, g:g + 1])
                            nc.scalar.activation(
                                dt[g][:, hh * LH:(hh + 1) * LH], edt[:],
                                AF.Ln, bias=1.0)
                        # du/sk per half right after its dt lands
                        for g in range(NG):
                            hs = slice(hh * LH, (hh + 1) * LH)
                            nc.vector.tensor_tensor(out=du[g][:, hs],
                                                    in0=dt[g][:, hs],
                                                    in1=u[g][:, hs],
                                                    op=OP.mult)
                            nc.vector.tensor_scalar(
                                sk[g][:, hs], u[g][:, hs],
                                dskip_sb[:, g:g + 1], None, OP.mult)
                # z (own half) -> silu -> z[g]; only needed by the
                # pass-end gates, so emitted after the dt/du chain
                for g in range(NG):
                    for nn in range(NC512):
                        ps = psA.tile([128, 512], F32, tag="mmA", name="mmA")
                        for k in range(4):
                            nc.tensor.matmul(
                                out=ps[:],
                                lhsT=wz_sb[k][:, g * 128:(g + 1) * 128],
                                rhs=xpad_sb[k][:, DCONV - 1 + nn * 512:
                                               DCONV - 1 + (nn + 1) * 512],
                                start=(k == 0), stop=(k == 3))
                        nc.scalar.activation(
                            z[g][:, nn * 512:(nn + 1) * 512], ps[:],
                            AF.Silu)
            adctx.close()  # frees the u tiles before phase E

            # ---------------- Phase E: selective scan ----------------
            with ExitStack() as ectx:
                trb = ectx.enter_context(tc.tile_pool(name="trb", bufs=1))
                # dA double-buffered so the Act exps prefetch a full
                # (g, state-quad) iteration ahead of the DVE scan chain
                dApool = ectx.enter_context(tc.tile_pool(name="dAp",
                                                         bufs=2))
                psY = ectx.enter_context(
                    tc.tile_pool(name="psY", bufs=1, space="PSUM"))
                psF = ectx.enter_context(
                    tc.tile_pool(name="psF", bufs=4, space="PSUM"))
                osb_pool = ectx.enter_context(tc.tile_pool(name="osb",
                                                           bufs=4))

                first_pass_pre = bc_pre
                for h in range(NH):
                    hsl = slice(h * LH, (h + 1) * LH)
                    fearly = {}
                    for gp in range(NG // 2):
                        gs = [2 * gp, 2 * gp + 1]
                        # B/C quad tiles for this pass: the first two come
                        # either from the C/D-phase prefetch (very first
                        # pass) or are loaded now; later quads are loaded
                        # two ahead of the scan chain inside the np loop.
                        bct = first_pass_pre if first_pass_pre else {
                            np_: load_bc_np(h, np_) for np_ in range(2)}
                        first_pass_pre = None
                        ytiles = {g: [psY.tile([128, 512], F32,
                                               tag=f"psY{g % 2}_{q}",
                                               name=f"psY{g}_{h}_{q}")
                                      for q in range(2)] for g in gs}
                        # seed the fold chains with the D_skip*u term
                        for g in gs:
                            for q in range(2):
                                nc.tensor.matmul(
                                    out=ytiles[g][q][:],
                                    lhsT=ident[:],
                                    rhs=sk[g][:, h * LH + q * 512:
                                              h * LH + (q + 1) * 512],
                                    start=True, stop=False)

                        def w_mult(dst, g, bqt, eng):
                            eng.tensor_tensor(
                                out=dst[:].rearrange(
                                    "p (q l) -> p q l", q=QS),
                                in0=du[g][:, hsl].rearrange(
                                    "p (q l) -> p q l",
                                    q=1).to_broadcast([128, QS, LH]),
                                in1=bqt[:].rearrange(
                                    "p (q l) -> p q l", q=QS),
                                op=OP.mult)

                        for np_ in range(NQ):
                            if np_ + 2 < NQ:
                                bct[np_ + 2] = load_bc_np(h, np_ + 2)
                            for g in gs:
                                dA = dApool.tile([128, QS * LH], BF16,
                                                 tag="dAp", name="dA")
                                for qi in range(QS):
                                    n = np_ * QS + qi
                                    nidx = g * DS + n
                                    nc.scalar.activation(
                                        dA[:, qi * LH:(qi + 1) * LH],
                                        dt[g][:, hsl], AF.Exp,
                                        scale=A_sb[:, nidx:nidx + 1])
                                w_t = trb.tile([128, QS * LH], BF16,
                                               tag="w", name="w")
                                w_mult(w_t, g, bct[np_][0], nc.vector)
                                h_t = trb.tile([128, QS * LH], BF16,
                                               tag="h", name="h")
                                for qi in range(QS):
                                    n = np_ * QS + qi
                                    init = (0.0 if h == 0
                                            else carry[g][:, n:n + 1])
                                    nc.vector.tensor_tensor_scan(
                                        h_t[:, qi * LH:(qi + 1) * LH],
                                        dA[:, qi * LH:(qi + 1) * LH],
                                        w_t[:, qi * LH:(qi + 1) * LH],
                                        init, OP.mult, OP.add)
                                if h == 0:
                                    # save the carried state (last column
                                    # of each state's chunk)
                                    nc.scalar.copy(
                                        carry[g][:, np_ * QS:
                                                 (np_ + 1) * QS].rearrange(
                                            "p (q o) -> p q o", o=1),
                                        h_t[:].rearrange(
                                            "p (q l) -> p q l",
                                            q=QS)[:, :, LH - 1:LH])
                                p_t = trb.tile([128, QS * LH], BF16,
                                               tag="p", name="p")
                                nc.vector.tensor_tensor(
                                    out=p_t[:], in0=h_t[:],
                                    in1=bct[np_][1][:],
                                    op=OP.mult)
                                last = (np_ == NQ - 1)
                                for qi in range(QS):
                                    for q in range(2):
                                        nc.tensor.matmul(
                                            out=ytiles[g][q][:],
                                            lhsT=ident[:],
                                            rhs=p_t[:, qi * LH + q * 512:
                                                    qi * LH + (q + 1) * 512],
                                            start=False,
                                            stop=(last and qi == QS - 1))
                        # eviction + silu(z) gate for this pass's 2 groups
                        for g in gs:
                            for q in range(2):
                                nc.scalar.copy(
                                    sk[g][:, h * LH + q * 512:
                                          h * LH + (q + 1) * 512],
                                    ytiles[g][q][:])
                            nc.vector.tensor_tensor(
                                out=y_mm[g][:, hsl], in0=sk[g][:, hsl],
                                in1=z[g][:, hsl], op=OP.mult)
                        if h == NH - 1 and gp == 0:
                            for j in range(4):
                                tt = h * (LH // 128) + j
                                ps = psF.tile([128, D], F32,
                                              tag=f"fe{j}", name=f"fe{j}",
                                              bufs=1)
                                for g in (0, 1):
                                    nc.tensor.matmul(
                                        out=ps[:],
                                        lhsT=y_mm[g][:, tt * 128:
                                                     (tt + 1) * 128],
                                        rhs=wout_sb[:, g * D:(g + 1) * D],
                                        start=(g == 0), stop=False)
                                fearly[j] = ps

                    # ---- Phase F for this half: out_proj + scatter.
                    # On the final half, groups 0/1 of the first 4 tiles
                    # were pre-accumulated during the last scan pass, so
                    # only g2/g3 remain on the critical tail.
                    for j in range(L // NH // 128):
                        tt = h * (LH // 128) + j
                        if j in fearly:
                            ps = fearly.pop(j)
                            g_list = [2, 3]
                        else:
                            ps = psF.tile([128, D], F32, tag=f"fe{j % 4}",
                                          name=f"mmF{j % 4}", bufs=1)
                            g_list = list(range(NG))
                        for g in g_list:
                            nc.tensor.matmul(
                                out=ps[:],
                                lhsT=y_mm[g][:, tt * 128:(tt + 1) * 128],
                                rhs=wout_sb[:, g * D:(g + 1) * D],
                                start=(g == g_list[0] and len(g_list) == NG),
                                stop=(g == NG - 1))
                        o_sb = osb_pool.tile([128, D], BF16, tag="osb",
                                             name="osb")
                        nc.scalar.copy(o_sb[:], ps[:])
                        nc.gpsimd.indirect_dma_start(
                            out=out_bounce.opt(),
                            out_offset=bass.IndirectOffsetOnAxis(
                                ap=sidx_sb[:, tt:tt + 1], axis=0),
                            in_=o_sb[:],
                            in_offset=None)

            # -------- Phase G: ReduceScatter + LN/SiLU/residual --------
            with ExitStack() as gctx:
                phg2 = gctx.enter_context(tc.tile_pool(name="phg2",
                                                       bufs=1))
                # prefetch the residual/LN params before the collective so
                # their DMAs overlap the ReduceScatter wait
                xres_sb = phg2.tile([128, 4 * D], F32, tag="xres",
                                    name="xres")
                for qq in range(4):
                    nc.sync.dma_start(xres_sb[:, qq * D:(qq + 1) * D],
                                      xres_d.ap()[qq * 128:(qq + 1) * 128,
                                                  :])
                lng_sb = phg2.tile([128, D], BF16, tag="lng", name="lng")
                nc.sync.dma_start(lng_sb[:], ln_g_d.ap())
                lnb_sb = phg2.tile([128, D], BF16, tag="lnb", name="lnb")
                nc.sync.dma_start(lnb_sb[:], ln_b_d.ap())
                if for_timeline:
                    nc.sync.dma_start(rs_out[:], out_bounce[0:L // 4, :])
                else:
                    nc.gpsimd.collective_compute(
                        "ReduceScatter", OP.add, replica_groups=quad_groups,
                        ins=[out_bounce.opt()], outs=[rs_out.opt()])
                rs_bf = phg2.tile([128, 4 * D], BF16, tag="rsb",
                                  name="rsb")
                for qq in range(4):
                    nc.sync.dma_start(rs_bf[:, qq * D:(qq + 1) * D],
                                      rs_out[qq * 128:(qq + 1) * 128, :])

                for qq in range(4):
                    rtile = rs_bf[:, qq * D:(qq + 1) * D]
                    st = phg2.tile([128, 8], F32, tag=f"st{qq}",
                                   name=f"st{qq}")
                    nc.vector.bn_stats(st[:, 0:6], rtile)
                    nc.vector.bn_aggr(st[:, 6:8], st[:, 0:6])
                    mu = st[:, 6:7]
                    var = st[:, 7:8]
                    std = phg2.tile([128, 1], F32, tag=f"std{qq}",
                                    name=f"std{qq}")
                    nc.scalar.activation(std[:], var[:], AF.Sqrt,
                                         bias=eps_sb[:, 0:1])
                    rstd = phg2.tile([128, 1], F32, tag=f"rstd{qq}",
                                     name=f"rstd{qq}")
                    nc.vector.reciprocal(rstd[:], std[:])
                    nmr = phg2.tile([128, 1], F32, tag=f"nmr{qq}",
                                    name=f"nmr{qq}")
                    nc.vector.tensor_scalar(nmr[:], mu[:], rstd[:, 0:1],
                                            -1.0, OP.mult, OP.mult)
                    s1 = phg2.tile([128, D], BF16, tag=f"s1{qq}",
                                   name=f"s1{qq}")
                    nc.scalar.activation(s1[:], rtile, AF.Identity,
                                         scale=rstd[:, 0:1],
                                         bias=nmr[:, 0:1])
                    s2 = phg2.tile([128, D], BF16, tag=f"s2{qq}",
                                   name=f"s2{qq}")
                    nc.vector.tensor_tensor(out=s2[:], in0=s1[:],
                                            in1=lng_sb[:], op=OP.mult)
                    s3 = phg2.tile([128, D], BF16, tag=f"s3{qq}",
                                   name=f"s3{qq}")
                    nc.vector.tensor_tensor(out=s3[:], in0=s2[:],
                                            in1=lnb_sb[:], op=OP.add)
                    sil = phg2.tile([128, D], F32, tag=f"sil{qq}",
                                    name=f"sil{qq}")
                    nc.scalar.activation(sil[:], s3[:], AF.Silu)
                    fin = phg2.tile([128, D], F32, tag=f"fin{qq}",
                                    name=f"fin{qq}")
                    nc.vector.tensor_tensor(
                        out=fin[:], in0=sil[:],
                        in1=xres_sb[:, qq * D:(qq + 1) * D], op=OP.add)
                    nc.sync.dma_start(
                        out_d.ap()[qq * 128:(qq + 1) * 128, :], fin[:])

    _legalize_waits(nc)
    return nc


_NC_CACHE = {}


def _get_nc():
    if "nc" not in _NC_CACHE:
        _NC_CACHE["nc"] = _build_nc()
    return _NC_CACHE["nc"]


def _pg(a, ngroups):
    """[ngroups*128, k] -> [128, ngroups*k] partition-major regroup."""
    k = a.shape[1]
    return np.ascontiguousarray(
        a.reshape(ngroups, 128, k).transpose(1, 0, 2)).reshape(
            128, ngroups * k)


def _prep_in_maps(inputs):
    import ml_dtypes
    BF = ml_dtypes.bfloat16
    x = np.asarray(inputs["x"], np.float32)
    ln_g = np.asarray(inputs["ln_g"], np.float32)
    ln_b = np.asarray(inputs["ln_b"], np.float32)
    ln_gb = np.broadcast_to(ln_g[None, :], (128, D)).copy()
    ln_bb = np.broadcast_to(ln_b[None, :], (128, D)).copy()

    in_maps = []
    for c in range(NCORE):
        b, dr, half = c // 4, (c % 4) // 2, c % 2
        pfx = "fw_" if dr == 0 else "bw_"
        W_in = np.asarray(inputs[pfx + "W_in"], np.float32)
        convw = np.asarray(inputs[pfx + "conv_w"], np.float32)
        convb = np.asarray(inputs[pfx + "conv_b"], np.float32)
        W_x = np.asarray(inputs[pfx + "W_xproj"], np.float32)
        W_dt = np.asarray(inputs[pfx + "W_dt"], np.float32)
        b_dt = np.asarray(inputs[pfx + "b_dt"], np.float32)
        A_log = np.asarray(inputs[pfx + "A_log"], np.float32)
        dskip = np.asarray(inputs[pfx + "D_skip"], np.float32)
        W_out = np.asarray(inputs[pfx + "W_out"], np.float32)

        h0 = half * DH
        oh0 = (1 - half) * DH
        perm = np.concatenate([np.arange(h0, h0 + DH),
                               np.arange(oh0, oh0 + DH)])
        own = perm[:DH]

        xb = x[b] if dr == 0 else x[b][::-1]
        xpad = np.zeros((D, DCONV - 1 + L), np.float32)
        xpad[:, DCONV - 1:] = xb.T

        q = c % 4
        t = np.arange(NT * 128).reshape(NT, 128)
        rows = t if dr == 0 else (L - 1) - t

        consts = np.zeros((128, 128), np.float32)
        consts[:, 0:32] = _pg(convw[perm], NGF)
        consts[:, 32:40] = _pg(convb[perm][:, None], NGF)
        consts[:, 40:44] = _pg(b_dt[own][:, None], NG)
        consts[:, 44:108] = _pg(-np.exp(A_log[own]), NG)
        consts[:, 108:112] = _pg(dskip[own][:, None], NG)
        consts[:, 112] = 1e-5

        wx = np.empty((128, NGF * NPROJ), np.float32)
        for kg in range(NGF):
            ch = perm[kg * 128:(kg + 1) * 128]
            wx[:, kg * NPROJ:(kg + 1) * NPROJ] = W_x[:, ch].T

        m = {
            "xpad": xpad.astype(BF),
            "winT": np.ascontiguousarray(
                W_in[:DI][perm].T).astype(BF),
            "wz": np.ascontiguousarray(
                W_in[DI + h0:DI + h0 + DH].T).astype(BF),
            "wx": wx.astype(BF),
            "wdt": np.ascontiguousarray(W_dt[own].T).astype(BF),
            "wout": _pg(0.5 * W_out[:, own].T, NG).astype(BF),
            "consts": consts,
            "xres": np.ascontiguousarray(
                x[b][q * (L // 4):(q + 1) * (L // 4)]),
            "ln_g": ln_gb.astype(BF),
            "ln_b": ln_bb.astype(BF),
            "sidx": np.ascontiguousarray(rows.T.astype(np.int32)),
            "ident": np.eye(128, dtype=np.float32).astype(BF),
        }
        in_maps.append(m)
    return in_maps


def _assemble(results):
    out = np.empty((B, L, D), np.float32)
    for b in range(B):
        out[b] = np.concatenate(
            [results[4 * b + q]["out_shard"] for q in range(4)], axis=0)
    return out


def _run(inputs, trace=False, **kw):
    nc = _get_nc()
    in_maps = _prep_in_maps(inputs)
    res = bass_utils.run_bass_kernel_spmd(
        nc, in_maps, core_ids=list(range(NCORE)), trace=trace, **kw)
    return _assemble(res.results), res


def _timed_run(inputs, iters=10):
    """Run once for outputs, then time repeated executions of the jitted
    sharded body (no donation; inputs resident on device)."""
    import jax
    import numpy as _np
    from jax.sharding import Mesh, PartitionSpec, NamedSharding
    from jax.experimental.shard_map import shard_map
    import concourse.bass2jax as bass2jax
    import concourse.mybir as _mybir

    nc = _get_nc()
    in_maps = _prep_in_maps(inputs)
    bass2jax.install_neuronx_cc_hook()

    partition_name = (nc.partition_id_tensor.name
                      if nc.partition_id_tensor else None)
    in_names, out_names, out_avals, zero_outs = [], [], [], []
    for alloc in nc.m.functions[0].allocations:
        if not isinstance(alloc, _mybir.MemoryLocationSet):
            continue
        name = alloc.memorylocations[0].name
        if alloc.kind == "ExternalInput":
            if name != partition_name:
                in_names.append(name)
        elif alloc.kind == "ExternalOutput":
            shape = tuple(alloc.tensor_shape)
            dtype = _mybir.dt.np(alloc.dtype)
            out_names.append(name)
            out_avals.append(jax.core.ShapedArray(shape, dtype))
            zero_outs.append(_np.zeros(shape, dtype))
    n_params = len(in_names)
    n_outs = len(out_avals)
    all_in_names = list(in_names) + list(out_names)
    if partition_name is not None:
        all_in_names.append(partition_name)

    def _body(*args):
        operands = list(args)
        if partition_name is not None:
            operands.append(bass2jax.partition_id_tensor())
        outs = bass2jax._bass_exec_p.bind(
            *operands,
            out_avals=tuple(out_avals),
            in_names=tuple(all_in_names),
            out_names=tuple(out_names),
            lowering_input_output_aliases=(),
            sim_require_finite=True,
            sim_require_nnan=True,
            nc=nc,
        )
        return tuple(outs)

    devices = jax.devices()[:NCORE]
    mesh = Mesh(_np.asarray(devices), ("core",))
    in_specs = (PartitionSpec("core"),) * (n_params + n_outs)
    out_specs = (PartitionSpec("core"),) * n_outs
    donate = tuple(range(n_params, n_params + n_outs))
    sharded = jax.jit(
        shard_map(_body, mesh=mesh, in_specs=in_specs, out_specs=out_specs,
                  check_rep=False),
        donate_argnums=donate, keep_unused=True)

    sh = NamedSharding(mesh, PartitionSpec("core"))
    concat_in = [
        jax.device_put(_np.concatenate(
            [_np.asarray(in_maps[c][nm]) for c in range(NCORE)], axis=0), sh)
        for nm in in_names
    ]
    def make_zeros():
        return [jax.device_put(
            _np.zeros((NCORE * z.shape[0], *z.shape[1:]), z.dtype), sh)
            for z in zero_outs]

    jax.block_until_ready(concat_in)
    z0 = make_zeros()
    jax.block_until_ready(z0)
    out_arrs = jax.block_until_ready(sharded(*concat_in, *z0))
    results = [
        {nm: _np.asarray(out_arrs[i]).reshape(NCORE, *out_avals[i].shape)[c]
         for i, nm in enumerate(out_names)}
        for c in range(NCORE)
    ]
    output = _assemble(results)

    zsets = [make_zeros() for _ in range(iters)]
    jax.block_until_ready(zsets)
    best = None
    for zi in zsets:
        t1 = time.perf_counter()
        jax.block_until_ready(sharded(*concat_in, *zi))
        dt_s = time.perf_counter() - t1
        best = dt_s if best is None else min(best, dt_s)
    return output, int(best * 1e9)


def kernel(**inputs):
    out, _ = _run(inputs)
    return out
